# revision 1
# baseline (speedup 1.0000x reference)
"""Trainium2 Bass kernel for nn_NodeEdgeCrossAttention.

Strategy (dst-sharded, zero-collective):
  - Host sorts edges by destination node, assigns nodes to 8 cores with
    balanced padded-edge counts, and packs each node's edge run (padded to a
    multiple of 32) into 512-column chunks using a slot pattern shared by all
    cores (SPMD requires one program).  Each chunk holds at most 8 slots;
    slot s of chunk c gets global index c*8+s.
  - Scores fold Wq/Wk into per-node M matrices (score = M[dst] . k_raw), so
    no k-projection or q-gather is needed.  bk cancels by softmax shift
    invariance; bv folds through Wo into bo because sum(attn) == 1.
  - Per chunk: one fused kvs DMA (k | v | one-hot S), per-slot score matmuls,
    one exp, one DMA-transpose for edge-major exp values, 4 v-projection
    matmuls, one fused weighted-v multiply, and 4 segment matmuls with the
    one-hot S slot columns as weights accumulating [8 slots, 144] in PSUM
    (seg sums and softmax denominators together).  Park groups of 3 chunks
    drain to a DRAM scratch by DMA.
  - Numerics: fp16 for linear tensors, bf16 for exp-range tensors, fp32
    accumulation; validated at ~2e-3 max relative error.
"""

import numpy as np

N, E, DIM, HEADS = 10000, 640000, 128, 4
DH = DIM // HEADS
NCORES = 8
CHUNK = 512
TILE = 128
SCALE = DH ** -0.5
SP = 16              # exp staging columns per tile
PW = DIM + HEADS     # 132: per-tile rhs width (exv | exE)
GPC = 3              # chunks per PSUM park group


class Plan:
    pass


def _make_plan(dst):
    """Pack nodes into a chunk/slot layout shared across all 8 cores."""
    deg = np.bincount(dst, minlength=N)
    if deg.max() > 128:
        raise NotImplementedError(f"max degree {deg.max()} > 128 needs node splitting")
    Rn = np.maximum(np.ceil(deg / 32.0).astype(np.int64), 1) * 32

    order = np.argsort(-Rn, kind="stable")
    loads = np.zeros(NCORES, np.int64)
    core_nodes = [[] for _ in range(NCORES)]
    for n in order:
        c = int(loads.argmin())
        core_nodes[c].append(int(n))
        loads[c] += Rn[n]

    # Shared slot pattern = elementwise max over cores' (desc-sorted) R seqs.
    L = max(len(cn) for cn in core_nodes)
    pat = np.zeros(L, np.int64)
    for cn in core_nodes:
        r = Rn[np.array(cn, np.int64)]
        pat[: len(r)] = np.maximum(pat[: len(r)], r)

    slots = []           # {R, chunk, col0, pi}
    chunks = []          # {slots: [slot indices]}
    cur = {"slots": []}
    rem = CHUNK
    pi = 0
    while pi < L:
        R = int(pat[pi])
        if R <= rem:
            cur["slots"].append(len(slots))
            slots.append({"R": R, "chunk": len(chunks), "col0": CHUNK - rem, "pi": pi})
            rem -= R
            pi += 1
        else:
            if rem > 0:
                cur["slots"].append(len(slots))
                slots.append({"R": rem, "chunk": len(chunks),
                              "col0": CHUNK - rem, "pi": -1})
            chunks.append(cur)
            cur = {"slots": []}
            rem = CHUNK
    if rem > 0 and rem < CHUNK:
        cur["slots"].append(len(slots))
        slots.append({"R": rem, "chunk": len(chunks), "col0": CHUNK - rem, "pi": -1})
    if cur["slots"]:
        chunks.append(cur)

    max_ns = 0
    for ch in chunks:
        ch["ns"] = len(ch["slots"])
        max_ns = max(max_ns, ch["ns"])

    p = Plan()
    p.sl = max_ns                                    # slot positions per chunk
    p.kvw = 2 * CHUNK + 4 * p.sl
    p.deg = deg
    p.core_nodes = core_nodes
    p.slots = slots
    p.chunks = chunks
    p.nchunks = len(chunks)
    p.cols = p.nchunks * CHUNK
    p.nslot = p.nchunks * p.sl                       # sparse slot space
    p.nslot_b = ((p.nslot + TILE - 1) // TILE) * TILE    # 128-padded
    p.nsp = ((p.nslot + CHUNK - 1) // CHUNK) * CHUNK     # 512-padded
    return p


def _pack_core_inputs(plan, c, k_edges, v_edges, q_nodes, edges_of):
    """Per-core fused kvs [128, nchunks*KVW] f16, qT [128, nsp] f16, qslot."""
    import ml_dtypes
    cols = plan.cols
    edge_order = np.full(cols, -1, np.int64)
    qslot = np.full(plan.nslot, -1, np.int64)
    cn = plan.core_nodes[c]
    for ch_i, ch in enumerate(plan.chunks):
        for j, sidx in enumerate(ch["slots"]):
            s = plan.slots[sidx]
            if s["pi"] < 0 or s["pi"] >= len(cn):
                continue
            node = cn[s["pi"]]
            d = plan.deg[node]
            g0 = ch_i * CHUNK + s["col0"]
            edge_order[g0: g0 + d] = edges_of[node]
            qslot[ch_i * plan.sl + j] = node

    valid = edge_order >= 0
    idx = np.where(valid, edge_order, 0)
    kT = np.where(valid[:, None], k_edges[idx], 0.0).astype(np.float16).T
    vT = np.where(valid[:, None], v_edges[idx], 0.0).astype(np.float16).T

    # one-hot S: [128, nchunks*4*SLOTS], col (chunk, tile, slot_j)
    S = np.zeros((TILE, plan.nchunks * 4 * plan.sl), np.float32)
    for ci, ch in enumerate(plan.chunks):
        for j, sidx in enumerate(ch["slots"]):
            s = plan.slots[sidx]
            if s["pi"] < 0 or s["pi"] >= len(cn):
                continue
            d = int(plan.deg[cn[s["pi"]]])
            for t in range(4):
                lo = max(s["col0"], t * TILE)
                hi = min(s["col0"] + d, (t + 1) * TILE)
                if lo < hi:
                    S[lo - t * TILE:hi - t * TILE, (ci * 4 + t) * plan.sl + j] = 1.0
    Sbits = S.astype(ml_dtypes.bfloat16).view(np.float16)

    kvs = np.empty((TILE, plan.nchunks * plan.kvw), np.float16)
    kc = kT.reshape(TILE, plan.nchunks, CHUNK)
    vc = vT.reshape(TILE, plan.nchunks, CHUNK)
    sc = Sbits.reshape(TILE, plan.nchunks, 4 * plan.sl)
    kvw = kvs.reshape(TILE, plan.nchunks, plan.kvw)
    kvw[:, :, 0:CHUNK] = kc
    kvw[:, :, CHUNK:2 * CHUNK] = vc
    kvw[:, :, 2 * CHUNK:plan.kvw] = sc

    qvalid = qslot >= 0
    qidx = np.where(qvalid, qslot, 0)
    qT = np.zeros((DIM, plan.nsp), np.float16)
    qT[:, : plan.nslot] = np.where(qvalid[:, None], q_nodes[qidx], 0.0
                                   ).astype(np.float16).T
    return kvs, qT, qslot


# ---------------------------------------------------------------------------
# Device kernel emission
# ---------------------------------------------------------------------------

def _build_module(plan):
    import concourse.bacc as bacc
    import concourse.mybir as mybir
    import concourse.tile as tile
    from contextlib import ExitStack

    f16 = mybir.dt.float16
    bf = mybir.dt.bfloat16
    f32 = mybir.dt.float32
    NSP = plan.nsp
    NBLK = plan.nslot_b // TILE
    CW = PW              # 132 scratch row width
    SL = plan.sl
    KVW = plan.kvw

    nc = bacc.Bacc("TRN2", debug=False, num_devices=NCORES)

    kvs_d = nc.dram_tensor("kvs", [TILE, plan.nchunks * KVW], f16,
                           kind="ExternalInput")
    qT_d = nc.dram_tensor("qT", [DIM, NSP], f16, kind="ExternalInput")
    Wq_d = nc.dram_tensor("Wq", [DIM, DIM], f16, kind="ExternalInput")
    WkTs_d = nc.dram_tensor("WkTs", [DIM, DIM], f16, kind="ExternalInput")
    Wv_d = nc.dram_tensor("Wv", [DIM, DIM], f16, kind="ExternalInput")
    Wo_d = nc.dram_tensor("Wo", [DIM, DIM], f32, kind="ExternalInput")
    Hm_d = nc.dram_tensor("Hm", [DIM, HEADS], f16, kind="ExternalInput")
    ID_d = nc.dram_tensor("ID", [DIM, DIM], f32, kind="ExternalInput")
    I4_d = nc.dram_tensor("I4", [HEADS, HEADS], bf, kind="ExternalInput")
    bq_d = nc.dram_tensor("bq", [DIM, 1], f32, kind="ExternalInput")
    bo_d = nc.dram_tensor("bo", [DIM, 1], f32, kind="ExternalInput")
    accD = nc.dram_tensor("accD", [plan.nslot, CW], f32, kind="Internal")
    outT_d = nc.dram_tensor("outT", [DIM, NSP], f32, kind="ExternalOutput")

    Exp = mybir.ActivationFunctionType.Exp
    Ident = mybir.ActivationFunctionType.Identity
    mult = mybir.AluOpType.mult
    amax = mybir.AluOpType.max

    with ExitStack() as ctx:
        tc = ctx.enter_context(tile.TileContext(nc))
        cp = ctx.enter_context(tc.tile_pool(name="const", bufs=1))
        sp = ctx.enter_context(tc.tile_pool(name="persist", bufs=1))
        iop = ctx.enter_context(tc.tile_pool(name="io", bufs=4))
        xp = ctx.enter_context(tc.tile_pool(name="work", bufs=4))
        pp = ctx.enter_context(tc.tile_pool(name="ps", bufs=2, space="PSUM"))

        def dmac(tile_ap, dram_ap):
            nc.sync.dma_start(out=tile_ap, in_=dram_ap)

        Wq_sb = cp.tile([DIM, DIM], f16); dmac(Wq_sb[:], Wq_d[:, :])
        WkTs_sb = cp.tile([DIM, DIM], f16); dmac(WkTs_sb[:], WkTs_d[:, :])
        Wv_sb = cp.tile([DIM, DIM], f16); dmac(Wv_sb[:], Wv_d[:, :])
        Wo_sb = cp.tile([DIM, DIM], f32); dmac(Wo_sb[:], Wo_d[:, :])
        Hm_sb = cp.tile([DIM, HEADS], f16); dmac(Hm_sb[:], Hm_d[:, :])
        ID_sb = cp.tile([DIM, DIM], f32); dmac(ID_sb[:], ID_d[:, :])
        I4_sb = cp.tile([HEADS, HEADS], bf); dmac(I4_sb[:], I4_d[:, :])
        bq_sb = cp.tile([DIM, 1], f32); dmac(bq_sb[:], bq_d[:, :])
        bo_sb = cp.tile([DIM, 1], f32); dmac(bo_sb[:], bo_d[:, :])
        qT_sb = sp.tile([DIM, NSP], f16); dmac(qT_sb[:], qT_d[:, :])

        qp_sb = sp.tile([DIM, NSP], f16)
        M_sb = sp.tile([DIM, 4 * NSP], f16)

        # ---- Stage A: q projection + bias ----
        for b in range(NSP // CHUNK):
            sl = slice(b * CHUNK, (b + 1) * CHUNK)
            qp_ps = pp.tile([DIM, CHUNK], f32, tag="aux")
            nc.tensor.matmul(out=qp_ps[:], lhsT=Wq_sb[:], rhs=qT_sb[:, sl],
                             start=True, stop=True)
            nc.scalar.activation(out=qp_sb[:, sl], in_=qp_ps[:],
                                 func=Ident, bias=bq_sb[:, 0:1])

        # ---- Stage A: M matrices, 32 slots per group ----
        for g in range(NSP // 32):
            qsl = slice(g * 32, (g + 1) * 32)
            qpm = xp.tile([DIM, TILE], f16, tag="qpm")
            nc.vector.tensor_tensor(
                out=qpm[:].rearrange("p (w h) -> p w h", h=HEADS),
                in0=qp_sb[:, qsl].unsqueeze(-1).to_broadcast([DIM, 32, HEADS]),
                in1=Hm_sb[:, :].unsqueeze(1).to_broadcast([DIM, 32, HEADS]),
                op=mult)
            M_ps = pp.tile([DIM, TILE], f32, tag="aux")
            nc.tensor.matmul(out=M_ps[:], lhsT=WkTs_sb[:], rhs=qpm[:],
                             start=True, stop=True)
            nc.scalar.copy(out=M_sb[:, g * TILE:(g + 1) * TILE], in_=M_ps[:])

        # ---- Steady state ----
        park = None
        for ci, ch in enumerate(plan.chunks):
            kvt = iop.tile([TILE, KVW], f16, tag="kv")
            dmac(kvt[:], kvs_d[:, ci * KVW:(ci + 1) * KVW])
            kc = kvt[:, 0:CHUNK]
            vc = kvt[:, CHUNK:2 * CHUNK]
            Sc = kvt[:, 2 * CHUNK:KVW].bitcast(bf)

            score_ps = pp.tile([HEADS, CHUNK], f32, tag="score")
            for j, sidx in enumerate(ch["slots"]):
                s = plan.slots[sidx]
                g = ci * SL + j
                c0 = s["col0"]
                nc.tensor.matmul(
                    out=score_ps[0:HEADS, c0:c0 + s["R"]],
                    lhsT=M_sb[:, 4 * g:4 * g + 4],
                    rhs=kc[:, c0:c0 + s["R"]],
                    start=True, stop=True)

            ex_sb = xp.tile([HEADS, CHUNK], bf, tag="ex")
            nc.scalar.activation(out=ex_sb[:], in_=score_ps[:], func=Exp)
            exT_ps = pp.tile([TILE, 4 * HEADS], f32, tag="aux")
            for t in range(4):
                nc.tensor.matmul(
                    out=exT_ps[:, 4 * t:4 * t + 4],
                    lhsT=ex_sb[0:HEADS, t * TILE:(t + 1) * TILE],
                    rhs=I4_sb[:], start=True, stop=True)
            exE_sb = xp.tile([TILE, 4 * HEADS], bf, tag="exE")
            nc.scalar.copy(out=exE_sb[:], in_=exT_ps[:])

            vp_ps = pp.tile([TILE, CHUNK], f32, tag="vp")
            for t in range(4):
                nc.tensor.matmul(
                    out=vp_ps[:, t * TILE:(t + 1) * TILE],
                    lhsT=vc[:, t * TILE:(t + 1) * TILE],
                    rhs=Wv_sb[:], start=True, stop=True)

            exF_sb = xp.tile([TILE, 4 * PW], bf, tag="exF")
            exF_t = exF_sb[:].rearrange("p (t c) -> p t c", t=4)
            nc.vector.tensor_tensor(
                out=exF_t[:, :, 0:DIM].rearrange("p t (h d) -> p t h d", h=HEADS),
                in0=vp_ps[:].rearrange("p (t h d) -> p t h d", t=4, h=HEADS),
                in1=exE_sb[:].rearrange("p (t h) -> p t h", t=4)
                    .unsqueeze(-1).to_broadcast([TILE, 4, HEADS, DH]),
                op=mult)
            nc.scalar.copy(
                out=exF_t[:, :, DIM:PW],
                in_=exE_sb[:].rearrange("p (t h) -> p t h", t=4))

            gi = ci % GPC
            if gi == 0:
                park = pp.tile([SL, GPC * CW], f32, tag="park")
            for t in range(4):
                nc.tensor.matmul(
                    out=park[:, gi * CW:(gi + 1) * CW],
                    lhsT=Sc[:, t * SL:(t + 1) * SL],
                    rhs=exF_t[:, t, :],
                    start=(t == 0), stop=(t == 3))
            if gi == GPC - 1 or ci == plan.nchunks - 1:
                g0 = (ci // GPC) * GPC
                used = ci - g0 + 1
                stage = xp.tile([SL, GPC * CW], f32, tag="stage")
                nc.vector.tensor_copy(out=stage[:, 0:used * CW],
                                      in_=park[:, 0:used * CW])
                nc.scalar.dma_start(
                    out=accD[g0 * SL:(ci + 1) * SL, :]
                        .rearrange("(c j) w -> j c w", j=SL),
                    in_=stage[:, 0:used * CW]
                        .rearrange("j (c w) -> j c w", w=CW))

        # ---- Final: read scratch back aligned, normalize, project ----
        accR = sp.tile([TILE, NBLK * CW], f32)
        nc.gpsimd.memset(accR[:], 0.0)
        full = plan.nslot // TILE          # whole 128-row blocks
        if full:
            dmac(accR[:, 0:full * CW]
                 .rearrange("p (b w) -> p b w", w=CW),
                 accD[0:full * TILE, :].rearrange("(b p) w -> p b w", p=TILE))
        tail = plan.nslot - full * TILE
        if tail:
            dmac(accR[0:tail, full * CW:(full + 1) * CW],
                 accD[full * TILE:plan.nslot, :])

        rden_sb = sp.tile([TILE, NBLK * HEADS], f32)
        nc.vector.tensor_scalar(
            out=rden_sb[:].rearrange("p (b h) -> p b h", h=HEADS),
            in0=accR[:].rearrange("p (b w) -> p b w", w=CW)[:, :, DIM:DIM + HEADS],
            scalar1=1e-30, scalar2=None, op0=amax)
        nc.vector.reciprocal(out=rden_sb[:], in_=rden_sb[:])
        agg_sb = sp.tile([TILE, NBLK * DIM], f32)
        nc.vector.tensor_tensor(
            out=agg_sb[:].rearrange("p (b h d) -> p b h d", b=NBLK, h=HEADS),
            in0=accR[:].rearrange("p (b w) -> p b w", w=CW)[:, :, 0:DIM]
                .rearrange("p b (h d) -> p b h d", h=HEADS),
            in1=rden_sb[:].rearrange("p (b h) -> p b h", h=HEADS)
                .unsqueeze(-1).to_broadcast([TILE, NBLK, HEADS, DH]),
            op=mult)
        aggT_sb = sp.tile([TILE, NSP], f32)
        nc.gpsimd.memset(aggT_sb[:], 0.0)
        for b in range(NBLK):
            tp_ps = pp.tile([DIM, TILE], f32, tag="aux")
            nc.tensor.transpose(out=tp_ps[:],
                                in_=agg_sb[:, b * TILE:(b + 1) * TILE],
                                identity=ID_sb[:])
            nc.scalar.copy(out=aggT_sb[:, b * TILE:(b + 1) * TILE], in_=tp_ps[:])
        for b in range(NSP // CHUNK):
            sl = slice(b * CHUNK, (b + 1) * CHUNK)
            out_ps = pp.tile([DIM, CHUNK], f32, tag="aux")
            nc.tensor.matmul(out=out_ps[:], lhsT=Wo_sb[:],
                             rhs=aggT_sb[:, sl], start=True, stop=True)
            osb = xp.tile([DIM, CHUNK], f32, tag="osb")
            nc.scalar.activation(out=osb[:], in_=out_ps[:],
                                 func=Ident, bias=bo_sb[:, 0:1])
            dmac(outT_d[:, sl], osb[:])

    nc.compile()
    return nc


# ---------------------------------------------------------------------------
# Entry point
# ---------------------------------------------------------------------------

def _prepare(inputs):
    q_nodes = np.asarray(inputs["q_nodes"], np.float32)
    k_edges = np.asarray(inputs["k_edges"], np.float32)
    v_edges = np.asarray(inputs["v_edges"], np.float32)
    Wq = np.asarray(inputs["Wq"], np.float32)
    bq = np.asarray(inputs["bq"], np.float32)
    Wk = np.asarray(inputs["Wk"], np.float32)
    Wv = np.asarray(inputs["Wv"], np.float32)
    bv = np.asarray(inputs["bv"], np.float32)
    Wo = np.asarray(inputs["Wo"], np.float32)
    bo = np.asarray(inputs["bo"], np.float32)
    dst = np.asarray(inputs["edge_index"])[0].astype(np.int64)

    plan = _make_plan(dst)

    eorder = np.argsort(dst, kind="stable")
    starts = np.zeros(N + 1, np.int64)
    np.cumsum(np.bincount(dst, minlength=N), out=starts[1:])
    edges_of = [eorder[starts[n]: starts[n + 1]] for n in range(N)]

    consts = {
        "Wq": Wq.astype(np.float16),
        "WkTs": np.ascontiguousarray((Wk * SCALE).T).astype(np.float16),
        "Wv": Wv.astype(np.float16),
        "Wo": np.ascontiguousarray(Wo),
        "Hm": (np.arange(DIM)[:, None] // DH == np.arange(HEADS)[None, :]
               ).astype(np.float16),
        "ID": np.eye(DIM, dtype=np.float32),
        "I4": np.eye(HEADS).astype(__import__('ml_dtypes').bfloat16),
        "bq": bq.reshape(DIM, 1).astype(np.float32),
        # sum(attn)==1 folds bv through Wo: out = (segv/den)@Wo + (bv@Wo + bo)
        "bo": (bv @ Wo + bo).reshape(DIM, 1).astype(np.float32),
    }
    return plan, dst, edges_of, consts, q_nodes, k_edges, v_edges, bo


def kernel(**inputs):
    from concourse.bass_utils import run_bass_kernel_spmd

    (plan, dst, edges_of, consts, q_nodes, k_edges, v_edges, bo) = _prepare(inputs)

    nc = _build_module(plan)

    in_maps = []
    slot_maps = []
    for c in range(NCORES):
        kvs, qT, qslot = _pack_core_inputs(plan, c, k_edges, v_edges,
                                           q_nodes, edges_of)
        m = {"kvs": kvs, "qT": qT}
        m.update(consts)
        in_maps.append(m)
        slot_maps.append(qslot)

    res = run_bass_kernel_spmd(nc, in_maps, core_ids=list(range(NCORES)))
    global LAST_RESULTS
    LAST_RESULTS = res

    out = np.zeros((N, DIM), np.float32)
    for c in range(NCORES):
        outT = res.results[c]["outT"]          # [DIM, nsp]
        qslot = slot_maps[c]
        valid = qslot >= 0
        out[qslot[valid]] = outT[:, : plan.nslot].T[valid]
    deg0 = plan.deg == 0
    if deg0.any():
        out[deg0] = bo
    return out



# revision 13
# speedup vs baseline: 1.2122x; 1.2122x over previous
"""Trainium2 Bass kernel for nn_NodeEdgeCrossAttention.

Strategy (dst-sharded, zero-collective):
  - Host sorts edges by destination node, assigns nodes to 8 cores with
    balanced padded-edge counts (pad to 8), and bin-packs each node's edge
    run into 128-wide tiles (4 tiles per 512-col chunk) using a slot
    pattern shared by all cores (SPMD requires one program).  Filler slots
    complete every tile to exactly 128 rows so PSUM is fully written.
  - Scores are computed TRANSPOSED: per slot, lhsT = k columns (stationary),
    rhs = 4 M columns, so exp output lands edge-major with no transpose.
    M folds Wq/Wk/bq into per-head B_h matrices precomputed on host:
    M_h = B_h @ qT + c_h.  bk cancels by softmax shift invariance; bv folds
    through Wo into bo because sum(attn) == 1.
  - Per chunk: one fused kvs DMA (k | v | one-hot S), per-slot score
    matmuls into a shared PSUM tile (2 chunks per exp), one exp, 4
    v-projection matmuls, one weighted-v multiply (DVE), and 8 segment
    matmuls (4 weighted-v + 4 denominator) accumulating [SL, 132] in PSUM.
    Park groups of 3 chunks drain to a DRAM scratch by DMA.
  - Numerics: fp16 linear tensors, bf16 exp-range tensors, fp32 accumulate.
"""

import numpy as np

N, E, DIM, HEADS = 10000, 640000, 128, 4
DH = DIM // HEADS
NCORES = 8
CHUNK = 512
TILE = 128
TPC = CHUNK // TILE   # tiles (bins) per chunk
PAD = 8               # edge-run padding granularity
SCALE = DH ** -0.5
CW = DIM + HEADS      # 132: park row width (segv | denom)
GPC = 3               # chunks per PSUM park group
GEXP = 2              # chunks per exp activation


class Plan:
    pass


def _make_plan(dst):
    """Bin-pack nodes into a chunk/tile/slot layout shared across cores."""
    deg = np.bincount(dst, minlength=N)
    if deg.max() > TILE:
        raise NotImplementedError(f"max degree {deg.max()} > 128 needs node splitting")
    Rn = np.maximum(np.ceil(deg / float(PAD)).astype(np.int64), 1) * PAD

    # Round-robin by sorted rank: core c takes sorted[8i+c], so every core's
    # desc R sequence matches the shared pattern pat[i] = Rn[sorted[8i]].
    order = np.argsort(-Rn, kind="stable")
    core_nodes = [[int(n) for n in order[c::NCORES]] for c in range(NCORES)]
    L = (len(order) + NCORES - 1) // NCORES
    pat = Rn[order[::NCORES]]

    # First-fit-decreasing into 512-col chunks.  Within a chunk, items are
    # laid out sequentially; an item crossing a 128 tile boundary is split
    # into pieces that share one slot (seg matmuls accumulate over tiles).
    cks = []         # per chunk: list of (pi, start, R)
    space = []
    for pi in range(L):
        R = int(pat[pi])
        for b in range(len(cks)):
            if space[b] >= R:
                cks[b].append((pi, CHUNK - space[b], R))
                space[b] -= R
                break
        else:
            cks.append([(pi, 0, R)])
            space.append(CHUNK - R)
    for b in range(len(cks)):
        if space[b] > 0:
            cks[b].append((-1, CHUNK - space[b], space[b]))

    nchunks = len(cks)
    chunks = []
    for ci in range(nchunks):
        items = []    # (pi, slot j, start, R)
        pieces = []   # (slot j, tile, col0, r, ioff)
        for j, (pi, start, R) in enumerate(cks[ci]):
            items.append({"pi": pi, "j": j, "start": start, "R": R})
            col, rem, ioff = start, R, 0
            while rem > 0:
                t, c0 = col // TILE, col % TILE
                r = min(rem, TILE - c0)
                pieces.append({"j": j, "tile": t, "col0": c0, "r": r,
                               "ioff": ioff})
                col += r; rem -= r; ioff += r
        chunks.append({"items": items, "pieces": pieces})
    SL = max(len(ch["items"]) for ch in chunks)

    p = Plan()
    p.sl = SL
    p.kvw = 2 * CHUNK + TPC * SL
    p.deg = deg
    p.core_nodes = core_nodes
    p.chunks = chunks
    p.nchunks = nchunks
    p.cols = nchunks * CHUNK
    p.nslot = nchunks * SL
    p.nslot_b = ((p.nslot + TILE - 1) // TILE) * TILE
    p.nsp = ((p.nslot + CHUNK - 1) // CHUNK) * CHUNK
    return p


def _pack_core_inputs(plan, c, k_edges, v_edges, q_nodes, edges_of):
    """Per-core fused kvs [128, nchunks*KVW] f16, qT [128, nsp] f16, qslot."""
    import ml_dtypes
    cols = plan.cols
    edge_order = np.full(cols, -1, np.int64)
    qslot = np.full(plan.nslot, -1, np.int64)
    cn = plan.core_nodes[c]
    SL = plan.sl
    node_of = []     # per chunk: slot j -> node or -1
    for ci, ch in enumerate(plan.chunks):
        nmap = {}
        for it in ch["items"]:
            if it["pi"] < 0 or it["pi"] >= len(cn):
                continue
            node = cn[it["pi"]]
            nmap[it["j"]] = node
            d = plan.deg[node]
            g0 = ci * CHUNK + it["start"]
            edge_order[g0: g0 + d] = edges_of[node]
            qslot[ci * SL + it["j"]] = node
        node_of.append(nmap)

    valid = edge_order >= 0
    idx = np.where(valid, edge_order, 0)
    kT = np.where(valid[:, None], k_edges[idx], 0.0).astype(np.float16).T
    vT = np.where(valid[:, None], v_edges[idx], 0.0).astype(np.float16).T

    # one-hot S: [128, nchunks*TPC*SL], col (chunk, tile, slot_j)
    S = np.zeros((TILE, plan.nchunks * TPC * SL), np.float32)
    for ci, ch in enumerate(plan.chunks):
        nmap = node_of[ci]
        for pc in ch["pieces"]:
            node = nmap.get(pc["j"], -1)
            if node < 0:
                continue
            d = int(plan.deg[node])
            cov = min(d - pc["ioff"], pc["r"])
            if cov <= 0:
                continue
            col = (ci * TPC + pc["tile"]) * SL + pc["j"]
            S[pc["col0"]: pc["col0"] + cov, col] = 1.0
    Sbits = S.astype(ml_dtypes.bfloat16).view(np.float16)

    kvs = np.empty((TILE, plan.nchunks * plan.kvw), np.float16)
    kvw = kvs.reshape(TILE, plan.nchunks, plan.kvw)
    kvw[:, :, 0:CHUNK] = kT.reshape(TILE, plan.nchunks, CHUNK)
    kvw[:, :, CHUNK:2 * CHUNK] = vT.reshape(TILE, plan.nchunks, CHUNK)
    kvw[:, :, 2 * CHUNK:plan.kvw] = Sbits.reshape(TILE, plan.nchunks, TPC * SL)

    qvalid = qslot >= 0
    qidx = np.where(qvalid, qslot, 0)
    qT = np.zeros((DIM, plan.nsp), np.float16)
    qT[:, : plan.nslot] = np.where(qvalid[:, None], q_nodes[qidx], 0.0
                                   ).astype(np.float16).T
    return kvs, qT, qslot


# ---------------------------------------------------------------------------
# Device kernel emission
# ---------------------------------------------------------------------------

def _build_module(plan):
    import concourse.bacc as bacc
    import concourse.mybir as mybir
    import concourse.tile as tile
    from contextlib import ExitStack

    f16 = mybir.dt.float16
    bf = mybir.dt.bfloat16
    f32 = mybir.dt.float32
    NSP = plan.nsp
    NBLK = plan.nslot_b // TILE
    SL = plan.sl
    KVW = plan.kvw

    nc = bacc.Bacc("TRN2", debug=False, num_devices=NCORES)

    kvs_d = nc.dram_tensor("kvs", [TILE, plan.nchunks * KVW], f16,
                           kind="ExternalInput")
    qT_d = nc.dram_tensor("qT", [DIM, NSP], f16, kind="ExternalInput")
    BT4_d = nc.dram_tensor("BT4", [DIM, HEADS * DIM], f16, kind="ExternalInput")
    c4_d = nc.dram_tensor("c4", [DIM, HEADS], f32, kind="ExternalInput")
    Wv_d = nc.dram_tensor("Wv", [DIM, DIM], f16, kind="ExternalInput")
    Wo_d = nc.dram_tensor("Wo", [DIM, DIM], f32, kind="ExternalInput")
    ID_d = nc.dram_tensor("ID", [DIM, DIM], f32, kind="ExternalInput")
    bo_d = nc.dram_tensor("bo", [DIM, 1], f32, kind="ExternalInput")
    accD = nc.dram_tensor("accD", [plan.nslot, CW], f32, kind="Internal")
    outT_d = nc.dram_tensor("outT", [DIM, NSP], f32, kind="ExternalOutput")

    Exp = mybir.ActivationFunctionType.Exp
    Ident = mybir.ActivationFunctionType.Identity
    mult = mybir.AluOpType.mult
    amax = mybir.AluOpType.max

    with ExitStack() as ctx:
        tc = ctx.enter_context(tile.TileContext(nc))
        cp = ctx.enter_context(tc.tile_pool(name="const", bufs=1))
        sp = ctx.enter_context(tc.tile_pool(name="persist", bufs=1))
        iop = ctx.enter_context(tc.tile_pool(name="io", bufs=5))
        xp = ctx.enter_context(tc.tile_pool(name="work", bufs=3))
        pp = ctx.enter_context(tc.tile_pool(name="ps", bufs=2, space="PSUM"))
        ppv = ctx.enter_context(tc.tile_pool(name="psv", bufs=3, space="PSUM"))

        def dmac(tile_ap, dram_ap):
            nc.sync.dma_start(out=tile_ap, in_=dram_ap)

        BT4_sb = cp.tile([DIM, HEADS * DIM], f16); dmac(BT4_sb[:], BT4_d[:, :])
        c4_sb = cp.tile([DIM, HEADS], f32); dmac(c4_sb[:], c4_d[:, :])
        Wv_sb = cp.tile([DIM, DIM], f16); dmac(Wv_sb[:], Wv_d[:, :])
        Wo_sb = cp.tile([DIM, DIM], f32); dmac(Wo_sb[:], Wo_d[:, :])
        ID_sb = cp.tile([DIM, DIM], f32); dmac(ID_sb[:], ID_d[:, :])
        bo_sb = cp.tile([DIM, 1], f32); dmac(bo_sb[:], bo_d[:, :])
        qT_sb = sp.tile([DIM, NSP], f16); dmac(qT_sb[:], qT_d[:, :])

        # M4_sb col 4g+h = scaled WkT @ head-masked q-projection of slot g
        M4_sb = sp.tile([DIM, HEADS * NSP], f16)
        M4i = M4_sb[:].rearrange("p (g h) -> p g h", h=HEADS)

        # ---- Stage A: M = B_h @ qT + c_h, interleaved store ----
        for b in range(NSP // CHUNK):
            for h in range(HEADS):
                ps = ppv.tile([DIM, CHUNK], f32, tag="vp")
                nc.tensor.matmul(out=ps[:],
                                 lhsT=BT4_sb[:, h * DIM:(h + 1) * DIM],
                                 rhs=qT_sb[:, b * CHUNK:(b + 1) * CHUNK],
                                 start=True, stop=True)
                nc.scalar.activation(
                    out=M4i[:, b * CHUNK:(b + 1) * CHUNK, h],
                    in_=ps[:], func=Ident, bias=c4_sb[:, h:h + 1])

        # ---- Steady state ----
        park = None
        ex_ps = None
        group = []
        for ci, ch in enumerate(plan.chunks):
            kvt = iop.tile([TILE, KVW], f16, tag="kv")
            dmac(kvt[:], kvs_d[:, ci * KVW:(ci + 1) * KVW])
            kc = kvt[:, 0:CHUNK]

            gi = ci % GEXP
            if gi == 0:
                ex_ps = pp.tile([TILE, GEXP * 4 * HEADS], f32, tag="ex")
            # Matmul out base partition must be 32-aligned: extend each piece
            # down to alignment and issue per tile in reverse col order, so a
            # piece's extension rows (scored with the wrong M) are rewritten
            # by the owning piece, which executes later in PE program order.
            for pc in sorted(ch["pieces"],
                             key=lambda q: (q["tile"], -q["col0"])):
                g = ci * SL + pc["j"]
                c0 = pc["col0"]
                if c0 >= 64:
                    a0 = 64
                elif c0 >= 32 and c0 + pc["r"] <= 64:
                    a0 = 32
                else:
                    a0 = 0
                t = pc["tile"]
                nc.tensor.matmul(
                    out=ex_ps[a0:c0 + pc["r"],
                              gi * 4 * HEADS + 4 * t: gi * 4 * HEADS + 4 * t + 4],
                    lhsT=kc[:, t * TILE + a0: t * TILE + c0 + pc["r"]],
                    rhs=M4_sb[:, 4 * g:4 * g + 4],
                    start=True, stop=True)

            vp_ps = ppv.tile([TILE, CHUNK], f32, tag="vp")
            vc = kvt[:, CHUNK:2 * CHUNK]
            for t in range(TPC):
                nc.tensor.matmul(
                    out=vp_ps[:, t * TILE:(t + 1) * TILE],
                    lhsT=vc[:, t * TILE:(t + 1) * TILE],
                    rhs=Wv_sb[:], start=True, stop=True)
            group.append((ci, kvt, vp_ps, gi))
            if gi < GEXP - 1 and ci < plan.nchunks - 1:
                continue

            exE_sb = xp.tile([TILE, GEXP * 4 * HEADS], bf, tag="exE")
            nc.scalar.activation(out=exE_sb[:], in_=ex_ps[:], func=Exp)

            for (cj, kvtj, vpj, gj) in group:
                Sc = kvtj[:, 2 * CHUNK:KVW].bitcast(bf)
                exF_sb = xp.tile([TILE, CHUNK], bf, tag="exF")
                nc.vector.tensor_tensor(
                    out=exF_sb[:].rearrange("p (t h d) -> p t h d",
                                            t=TPC, h=HEADS),
                    in0=vpj[:].rearrange("p (t h d) -> p t h d",
                                         t=TPC, h=HEADS),
                    in1=exE_sb[:, gj * 4 * HEADS:(gj + 1) * 4 * HEADS]
                        .rearrange("p (t h) -> p t h", t=TPC)
                        .unsqueeze(-1).to_broadcast([TILE, TPC, HEADS, DH]),
                    op=mult)

                g2 = cj % GPC
                if g2 == 0:
                    park = pp.tile([SL, GPC * CW], f32, tag="park")
                base = g2 * CW
                for t in range(TPC):
                    nc.tensor.matmul(
                        out=park[:, base:base + DIM],
                        lhsT=Sc[:, t * SL:(t + 1) * SL],
                        rhs=exF_sb[:, t * TILE:(t + 1) * TILE],
                        start=(t == 0), stop=(t == TPC - 1))
                for t in range(TPC):
                    nc.tensor.matmul(
                        out=park[:, base + DIM:base + CW],
                        lhsT=Sc[:, t * SL:(t + 1) * SL],
                        rhs=exE_sb[:, gj * 4 * HEADS + 4 * t:
                                   gj * 4 * HEADS + 4 * t + 4],
                        start=(t == 0), stop=(t == TPC - 1))
                if g2 == GPC - 1 or cj == plan.nchunks - 1:
                    g0 = (cj // GPC) * GPC
                    used = cj - g0 + 1
                    stage = xp.tile([SL, GPC * CW], f32, tag="stage")
                    nc.vector.tensor_copy(out=stage[:, 0:used * CW],
                                          in_=park[:, 0:used * CW])
                    nc.scalar.dma_start(
                        out=accD[g0 * SL:(cj + 1) * SL, :]
                            .rearrange("(c j) w -> j c w", j=SL),
                        in_=stage[:, 0:used * CW]
                            .rearrange("j (c w) -> j c w", w=CW))
            group = []

        # ---- Final: read scratch back aligned, normalize, project ----
        accR = sp.tile([TILE, NBLK * CW], f32)
        nc.gpsimd.memset(accR[:], 0.0)
        full = plan.nslot // TILE
        if full:
            dmac(accR[:, 0:full * CW]
                 .rearrange("p (b w) -> p b w", w=CW),
                 accD[0:full * TILE, :].rearrange("(b p) w -> p b w", p=TILE))
        tail = plan.nslot - full * TILE
        if tail:
            dmac(accR[0:tail, full * CW:(full + 1) * CW],
                 accD[full * TILE:plan.nslot, :])

        rden_sb = sp.tile([TILE, NBLK * HEADS], f32)
        nc.vector.tensor_scalar(
            out=rden_sb[:].rearrange("p (b h) -> p b h", h=HEADS),
            in0=accR[:].rearrange("p (b w) -> p b w", w=CW)[:, :, DIM:DIM + HEADS],
            scalar1=1e-30, scalar2=None, op0=amax)
        nc.vector.reciprocal(out=rden_sb[:], in_=rden_sb[:])
        agg_sb = sp.tile([TILE, NBLK * DIM], f32)
        nc.vector.tensor_tensor(
            out=agg_sb[:].rearrange("p (b h d) -> p b h d", b=NBLK, h=HEADS),
            in0=accR[:].rearrange("p (b w) -> p b w", w=CW)[:, :, 0:DIM]
                .rearrange("p b (h d) -> p b h d", h=HEADS),
            in1=rden_sb[:].rearrange("p (b h) -> p b h", h=HEADS)
                .unsqueeze(-1).to_broadcast([TILE, NBLK, HEADS, DH]),
            op=mult)
        aggT_sb = sp.tile([TILE, NSP], f32)
        nc.gpsimd.memset(aggT_sb[:], 0.0)
        for b in range(NBLK):
            tp_ps = pp.tile([DIM, TILE], f32, tag="park")
            nc.tensor.transpose(out=tp_ps[:],
                                in_=agg_sb[:, b * TILE:(b + 1) * TILE],
                                identity=ID_sb[:])
            nc.scalar.copy(out=aggT_sb[:, b * TILE:(b + 1) * TILE], in_=tp_ps[:])
        for b in range(NSP // CHUNK):
            sl = slice(b * CHUNK, (b + 1) * CHUNK)
            out_ps = ppv.tile([DIM, CHUNK], f32, tag="vp")
            nc.tensor.matmul(out=out_ps[:], lhsT=Wo_sb[:],
                             rhs=aggT_sb[:, sl], start=True, stop=True)
            osb = xp.tile([DIM, CHUNK], f32, tag="osb")
            nc.scalar.activation(out=osb[:], in_=out_ps[:],
                                 func=Ident, bias=bo_sb[:, 0:1])
            dmac(outT_d[:, sl], osb[:])

    nc.compile()
    return nc


# ---------------------------------------------------------------------------
# Entry point
# ---------------------------------------------------------------------------

def _prepare(inputs):
    q_nodes = np.asarray(inputs["q_nodes"], np.float32)
    k_edges = np.asarray(inputs["k_edges"], np.float32)
    v_edges = np.asarray(inputs["v_edges"], np.float32)
    Wq = np.asarray(inputs["Wq"], np.float32)
    bq = np.asarray(inputs["bq"], np.float32)
    Wk = np.asarray(inputs["Wk"], np.float32)
    Wv = np.asarray(inputs["Wv"], np.float32)
    bv = np.asarray(inputs["bv"], np.float32)
    Wo = np.asarray(inputs["Wo"], np.float32)
    bo = np.asarray(inputs["bo"], np.float32)
    dst = np.asarray(inputs["edge_index"])[0].astype(np.int64)

    plan = _make_plan(dst)

    eorder = np.argsort(dst, kind="stable")
    starts = np.zeros(N + 1, np.int64)
    np.cumsum(np.bincount(dst, minlength=N), out=starts[1:])
    edges_of = [eorder[starts[n]: starts[n + 1]] for n in range(N)]

    # M_h = B_h @ qT + c_h with B_h = WkS diag(Hm_h) Wq^T, so score columns
    # match score = (Wk*scale)^T k . (Hm_h * (Wq^T q + bq)).
    WkS = Wk * SCALE
    Hm = (np.arange(DIM)[:, None] // DH == np.arange(HEADS)[None, :])
    BT4 = np.empty((DIM, HEADS * DIM), np.float32)
    c4 = np.empty((DIM, HEADS), np.float32)
    for h in range(HEADS):
        m = Hm[:, h].astype(np.float32)
        BT4[:, h * DIM:(h + 1) * DIM] = (Wq * m[None, :]) @ WkS.T
        c4[:, h] = WkS @ (m * bq)

    consts = {
        "BT4": BT4.astype(np.float16),
        "c4": c4,
        "Wv": Wv.astype(np.float16),
        "Wo": np.ascontiguousarray(Wo),
        "ID": np.eye(DIM, dtype=np.float32),
        # sum(attn)==1 folds bv through Wo: out = (segv/den)@Wo + (bv@Wo + bo)
        "bo": (bv @ Wo + bo).reshape(DIM, 1).astype(np.float32),
    }
    return plan, dst, edges_of, consts, q_nodes, k_edges, v_edges, bo


def kernel(**inputs):
    from concourse.bass_utils import run_bass_kernel_spmd

    (plan, dst, edges_of, consts, q_nodes, k_edges, v_edges, bo) = _prepare(inputs)

    nc = _build_module(plan)

    in_maps = []
    slot_maps = []
    for c in range(NCORES):
        kvs, qT, qslot = _pack_core_inputs(plan, c, k_edges, v_edges,
                                           q_nodes, edges_of)
        m = {"kvs": kvs, "qT": qT}
        m.update(consts)
        in_maps.append(m)
        slot_maps.append(qslot)

    res = run_bass_kernel_spmd(nc, in_maps, core_ids=list(range(NCORES)))
    global LAST_RESULTS
    LAST_RESULTS = res

    out = np.zeros((N, DIM), np.float32)
    for c in range(NCORES):
        outT = res.results[c]["outT"]          # [DIM, nsp]
        qslot = slot_maps[c]
        valid = qslot >= 0
        out[qslot[valid]] = outT[:, : plan.nslot].T[valid]
    deg0 = plan.deg == 0
    if deg0.any():
        out[deg0] = bo
    return out


# revision 17
# speedup vs baseline: 1.5840x; 1.3067x over previous
"""Trainium2 Bass kernel for nn_NodeEdgeCrossAttention.

Strategy (dst-sharded, zero-collective, whole-tile matmuls):
  - Host sorts edges by destination node, assigns nodes to 8 cores round-
    robin by sorted degree rank so all cores share one slot pattern (SPMD
    requires one program), and packs each node's edge run (no padding)
    into 512-col chunks; runs may cross the 128-row tile boundaries.
  - Per chunk, only 8 PE instructions, all with dense 128-wide operands:
      4 score matmuls  ex[e,(h,slot)] = k_tile^T @ M_cols   (one per tile)
      4 U matmuls      U[(h,slot), d|1] = exM_tile^T @ [v_edge-major|ones]
    exp runs on ACT over the full score tile; DVE masks it by the one-hot
    S to give exM = attn-weighted one-hot (wrong-slot entries zeroed), so
    the U matmul both aggregates raw v per (head, slot) and accumulates
    the softmax denominator through the trailing ones column.
  - M folds Wq/Wk/bq per head: M_h = B_h @ qT + c_h (host-precomputed
    B_h, c_h).  bk cancels by softmax shift invariance.  Wv never touches
    edge data: out = sum_h (U_h/den_h) @ P_h + bo', with P_h = Wv[:,h]Wo[h,:]
    and bo' = bv@Wo + bo (sum(attn)==1).
  - Park groups of 3 chunks accumulate U in PSUM, drain via a DRAM scratch
    keyed [head][slot] so the final normalize/transpose/project runs per
    head with plain affine access patterns.
  - Numerics: fp16 k/q/M, bf16 v/exp tensors, fp32 accumulation.
"""

import numpy as np

N, E, DIM, HEADS = 10000, 640000, 128, 4
DH = DIM // HEADS
NCORES = 8
CHUNK = 512
TILE = 128
TPC = CHUNK // TILE
SCALE = DH ** -0.5
VW = TILE + 1          # 129: v-block width per tile (v | ones)
GPC = 3                # chunks per PSUM park group
GEXP = 2               # chunks per exp activation
CAP = 12               # max items per chunk (bounds SL)


class Plan:
    pass


def _make_plan(dst):
    """Pack nodes into a shared chunk/slot layout; no edge padding."""
    deg = np.bincount(dst, minlength=N)
    if deg.max() > CHUNK:
        raise NotImplementedError(f"max degree {deg.max()} > {CHUNK}")
    live = np.nonzero(deg > 0)[0]
    Rl = deg[live].astype(np.int64)

    # Round-robin by sorted rank: core c takes sorted[8i+c]; shared pattern
    # pat[i] = max degree in rank octet i = degree of sorted[8i].
    order = np.argsort(-Rl, kind="stable")
    core_nodes = [[int(live[n]) for n in order[c::NCORES]]
                  for c in range(NCORES)]
    pat = Rl[order[::NCORES]]
    L = len(pat)

    # First-fit-decreasing into 512-col chunks (item count capped).
    cks = []
    space = []
    for pi in range(L):
        R = int(pat[pi])
        for b in range(len(cks)):
            if space[b] >= R and len(cks[b]) < CAP:
                cks[b].append((pi, CHUNK - space[b], R))
                space[b] -= R
                break
        else:
            cks.append([(pi, 0, R)])
            space.append(CHUNK - R)

    nchunks = len(cks)
    chunks = []
    for ci in range(nchunks):
        items = []
        pieces = []   # host-side S construction only
        for j, (pi, start, R) in enumerate(cks[ci]):
            items.append({"pi": pi, "j": j, "start": start, "R": R})
            col, rem, ioff = start, R, 0
            while rem > 0:
                t, c0 = col // TILE, col % TILE
                r = min(rem, TILE - c0)
                pieces.append({"j": j, "tile": t, "col0": c0, "r": r,
                               "ioff": ioff})
                col += r; rem -= r; ioff += r
        chunks.append({"items": items, "pieces": pieces})
    SL = max(len(ch["items"]) for ch in chunks)

    p = Plan()
    p.sl = SL
    p.kvw = CHUNK + TPC * VW + TPC * SL
    p.deg = deg
    p.core_nodes = core_nodes
    p.chunks = chunks
    p.nchunks = nchunks
    p.cols = nchunks * CHUNK
    p.nslot = nchunks * SL
    p.nslot_b = ((p.nslot + TILE - 1) // TILE) * TILE
    p.nsp = ((p.nslot + CHUNK - 1) // CHUNK) * CHUNK
    return p


def _pack_core_inputs(plan, c, k_edges, v_edges, q_nodes, edges_of):
    """Per-core fused kvs [128, nchunks*KVW] f16, qT [128, nsp] f16, qslot."""
    import ml_dtypes
    bf16 = ml_dtypes.bfloat16
    cols = plan.cols
    edge_order = np.full(cols, -1, np.int64)
    qslot = np.full(plan.nslot, -1, np.int64)
    cn = plan.core_nodes[c]
    SL = plan.sl
    node_of = []
    for ci, ch in enumerate(plan.chunks):
        nmap = {}
        for it in ch["items"]:
            if it["pi"] < 0 or it["pi"] >= len(cn):
                continue
            node = cn[it["pi"]]
            nmap[it["j"]] = node
            d = plan.deg[node]
            g0 = ci * CHUNK + it["start"]
            edge_order[g0: g0 + d] = edges_of[node]
            qslot[ci * SL + it["j"]] = node
        node_of.append(nmap)

    valid = edge_order >= 0
    idx = np.where(valid, edge_order, 0)
    kT = np.where(valid[:, None], k_edges[idx], 0.0).astype(np.float16).T
    vE = np.where(valid[:, None], v_edges[idx], 0.0).astype(bf16)

    # edge-major v blocks with a trailing ones column per tile
    vem = np.zeros((TILE, plan.nchunks, TPC, VW), bf16)
    vem[:, :, :, 0:TILE] = vE.reshape(plan.nchunks, TPC, TILE, DIM
                                      ).transpose(2, 0, 1, 3)
    vem[:, :, :, TILE] = bf16(1.0)

    # one-hot S: [128, nchunks*TPC*SL], col (chunk, tile, slot_j)
    S = np.zeros((TILE, plan.nchunks * TPC * SL), np.float32)
    for ci, ch in enumerate(plan.chunks):
        nmap = node_of[ci]
        for pc in ch["pieces"]:
            node = nmap.get(pc["j"], -1)
            if node < 0:
                continue
            d = int(plan.deg[node])
            cov = min(d - pc["ioff"], pc["r"])
            if cov <= 0:
                continue
            col = (ci * TPC + pc["tile"]) * SL + pc["j"]
            S[pc["col0"]: pc["col0"] + cov, col] = 1.0
    Sbits = S.astype(bf16).view(np.float16)

    kvs = np.empty((TILE, plan.nchunks * plan.kvw), np.float16)
    kvw = kvs.reshape(TILE, plan.nchunks, plan.kvw)
    kvw[:, :, 0:CHUNK] = kT.reshape(TILE, plan.nchunks, CHUNK)
    kvw[:, :, CHUNK:CHUNK + TPC * VW] = vem.reshape(
        TILE, plan.nchunks, TPC * VW).view(np.float16)
    kvw[:, :, CHUNK + TPC * VW:] = Sbits.reshape(TILE, plan.nchunks, TPC * SL)

    qvalid = qslot >= 0
    qidx = np.where(qvalid, qslot, 0)
    qT = np.zeros((DIM, plan.nsp), np.float16)
    qT[:, : plan.nslot] = np.where(qvalid[:, None], q_nodes[qidx], 0.0
                                   ).astype(np.float16).T
    return kvs, qT, qslot


# ---------------------------------------------------------------------------
# Device kernel emission
# ---------------------------------------------------------------------------

def _build_module(plan):
    import concourse.bacc as bacc
    import concourse.mybir as mybir
    import concourse.tile as tile
    from contextlib import ExitStack

    f16 = mybir.dt.float16
    bf = mybir.dt.bfloat16
    f32 = mybir.dt.float32
    NSP = plan.nsp
    NBLK = plan.nslot_b // TILE
    SL = plan.sl
    SW = TPC * SL          # score cols per chunk... per tile: 4*SL? no: HEADS*SL
    HS = HEADS * SL        # score cols per tile (h-major: h*SL+s)
    KVW = plan.kvw
    NCH = plan.nchunks
    UW = VW                # 129 park row width per chunk (U | den)

    nc = bacc.Bacc("TRN2", debug=False, num_devices=NCORES)

    kvs_d = nc.dram_tensor("kvs", [TILE, NCH * KVW], f16,
                           kind="ExternalInput")
    qT_d = nc.dram_tensor("qT", [DIM, NSP], f16, kind="ExternalInput")
    BT4_d = nc.dram_tensor("BT4", [DIM, HEADS * DIM], f16, kind="ExternalInput")
    c4_d = nc.dram_tensor("c4", [DIM, HEADS], f32, kind="ExternalInput")
    P4_d = nc.dram_tensor("P4", [DIM, HEADS * DIM], f32, kind="ExternalInput")
    ID_d = nc.dram_tensor("ID", [DIM, DIM], f32, kind="ExternalInput")
    bo_d = nc.dram_tensor("bo", [DIM, 1], f32, kind="ExternalInput")
    accD = nc.dram_tensor("accD", [HEADS, plan.nslot, UW], f32, kind="Internal")
    outT_d = nc.dram_tensor("outT", [DIM, NSP], f32, kind="ExternalOutput")

    Exp = mybir.ActivationFunctionType.Exp
    Ident = mybir.ActivationFunctionType.Identity
    mult = mybir.AluOpType.mult
    amax = mybir.AluOpType.max

    with ExitStack() as ctx:
        tc = ctx.enter_context(tile.TileContext(nc))
        cp = ctx.enter_context(tc.tile_pool(name="const", bufs=1))
        sp = ctx.enter_context(tc.tile_pool(name="persist", bufs=1))
        iop = ctx.enter_context(tc.tile_pool(name="io", bufs=6))
        xp = ctx.enter_context(tc.tile_pool(name="work", bufs=3))
        pp = ctx.enter_context(tc.tile_pool(name="ps", bufs=2, space="PSUM"))
        ppv = ctx.enter_context(tc.tile_pool(name="psv", bufs=3, space="PSUM"))

        def dmac(tile_ap, dram_ap):
            nc.sync.dma_start(out=tile_ap, in_=dram_ap)

        BT4_sb = cp.tile([DIM, HEADS * DIM], f16); dmac(BT4_sb[:], BT4_d[:, :])
        c4_sb = cp.tile([DIM, HEADS], f32); dmac(c4_sb[:], c4_d[:, :])
        P4_sb = cp.tile([DIM, HEADS * DIM], f32); dmac(P4_sb[:], P4_d[:, :])
        ID_sb = cp.tile([DIM, DIM], f32); dmac(ID_sb[:], ID_d[:, :])
        bo_sb = cp.tile([DIM, 1], f32); dmac(bo_sb[:], bo_d[:, :])
        qT_sb = sp.tile([DIM, NSP], f16); dmac(qT_sb[:], qT_d[:, :])

        # M4_sb per chunk block: col ci*HS + h*SL + j  (score-all rhs order)
        M4_sb = sp.tile([DIM, NCH * HS], f16)
        M4r = M4_sb[:].rearrange("p (c h s) -> p c h s", h=HEADS, s=SL)

        # ---- Stage A: M = B_h @ qT + c_h over chunk-aligned blocks ----
        CG = CHUNK // SL                 # chunks per stage-A block
        for cb in range(0, NCH, CG):
            ce = min(cb + CG, NCH)
            w = (ce - cb) * SL
            for h in range(HEADS):
                ps = ppv.tile([DIM, CHUNK], f32, tag="vp")
                nc.tensor.matmul(out=ps[:, 0:w],
                                 lhsT=BT4_sb[:, h * DIM:(h + 1) * DIM],
                                 rhs=qT_sb[:, cb * SL: cb * SL + w],
                                 start=True, stop=True)
                nc.scalar.activation(
                    out=M4r[:, cb:ce, h, :],
                    in_=ps[:, 0:w].rearrange("p (c s) -> p c s", s=SL),
                    func=Ident, bias=c4_sb[:, h:h + 1])

        # ---- Steady state ----
        park = None
        ex_ps = None
        group = []
        for ci, ch in enumerate(plan.chunks):
            kvt = iop.tile([TILE, KVW], f16, tag="kv")
            dmac(kvt[:], kvs_d[:, ci * KVW:(ci + 1) * KVW])
            kc = kvt[:, 0:CHUNK]

            gi = ci % GEXP
            if gi == 0:
                ex_ps = pp.tile([TILE, GEXP * TPC * HS], f32, tag="ex")
            for t in range(TPC):
                nc.tensor.matmul(
                    out=ex_ps[:, (gi * TPC + t) * HS:(gi * TPC + t + 1) * HS],
                    lhsT=kc[:, t * TILE:(t + 1) * TILE],
                    rhs=M4_sb[:, ci * HS:(ci + 1) * HS],
                    start=True, stop=True)
            group.append((ci, kvt, gi))
            if gi < GEXP - 1 and ci < plan.nchunks - 1:
                continue

            exE_sb = xp.tile([TILE, GEXP * TPC * HS], bf, tag="exE")
            nc.scalar.activation(out=exE_sb[:], in_=ex_ps[:], func=Exp)

            for (cj, kvtj, gj) in group:
                Sc = kvtj[:, CHUNK + TPC * VW:KVW].bitcast(bf)
                vem = kvtj[:, CHUNK:CHUNK + TPC * VW].bitcast(bf)
                exM_sb = xp.tile([TILE, TPC * HS], bf, tag="exM")
                nc.vector.tensor_tensor(
                    out=exM_sb[:].rearrange("p (t h s) -> p t h s",
                                            t=TPC, h=HEADS),
                    in0=exE_sb[:, gj * TPC * HS:(gj + 1) * TPC * HS]
                        .rearrange("p (t h s) -> p t h s", t=TPC, h=HEADS),
                    in1=Sc[:].rearrange("p (t s) -> p t s", t=TPC)
                        .unsqueeze(2).to_broadcast([TILE, TPC, HEADS, SL]),
                    op=mult)

                g2 = cj % GPC
                if g2 == 0:
                    park = pp.tile([HS, GPC * UW], f32, tag="park")
                for t in range(TPC):
                    nc.tensor.matmul(
                        out=park[:, g2 * UW:(g2 + 1) * UW],
                        lhsT=exM_sb[:, t * HS:(t + 1) * HS],
                        rhs=vem[:, t * VW:(t + 1) * VW],
                        start=(t == 0), stop=(t == TPC - 1))
                if g2 == GPC - 1 or cj == plan.nchunks - 1:
                    g0 = (cj // GPC) * GPC
                    used = cj - g0 + 1
                    stage = xp.tile([HS, GPC * UW], f32, tag="stage")
                    nc.scalar.copy(out=stage[:, 0:used * UW],
                                   in_=park[:, 0:used * UW])
                    for h in range(HEADS):
                        nc.scalar.dma_start(
                            out=accD[h, g0 * SL:(cj + 1) * SL, :]
                                .rearrange("(c j) w -> j c w", j=SL),
                            in_=stage[h * SL:(h + 1) * SL, 0:used * UW]
                                .rearrange("j (c w) -> j c w", w=UW))
            group = []

        # ---- Final: per head, read back, normalize, transpose, project ----
        accR = sp.tile([TILE, HEADS * NBLK * UW], f32)
        nc.gpsimd.memset(accR[:], 0.0)
        full = plan.nslot // TILE
        tail = plan.nslot - full * TILE
        for h in range(HEADS):
            base = h * NBLK * UW
            if full:
                dmac(accR[:, base:base + full * UW]
                     .rearrange("p (b w) -> p b w", w=UW),
                     accD[h, 0:full * TILE, :]
                     .rearrange("(b p) w -> p b w", p=TILE))
            if tail:
                dmac(accR[0:tail, base + full * UW:base + (full + 1) * UW],
                     accD[h, full * TILE:plan.nslot, :])

        accR4 = accR[:].rearrange("p (h b w) -> p h b w", h=HEADS, w=UW)
        rden_sb = sp.tile([TILE, HEADS * NBLK], f32)
        rden4 = rden_sb[:].rearrange("p (h b) -> p h b", h=HEADS)
        nc.vector.tensor_scalar(
            out=rden4.unsqueeze(-1),
            in0=accR4[:, :, :, TILE:TILE + 1],
            scalar1=1e-30, scalar2=None, op0=amax)
        nc.vector.reciprocal(out=rden_sb[:], in_=rden_sb[:])
        uN_sb = sp.tile([TILE, HEADS * NBLK * DIM], f32)
        nc.vector.tensor_tensor(
            out=uN_sb[:].rearrange("p (h b d) -> p h b d", h=HEADS, d=DIM),
            in0=accR4[:, :, :, 0:TILE],
            in1=rden4.unsqueeze(-1).to_broadcast([TILE, HEADS, NBLK, DIM]),
            op=mult)
        uT_sb = sp.tile([TILE, HEADS * NSP], f32)
        nc.gpsimd.memset(uT_sb[:], 0.0)
        for h in range(HEADS):
            for b in range(NBLK):
                tp_ps = pp.tile([DIM, TILE], f32, tag="park")
                nc.tensor.transpose(
                    out=tp_ps[:],
                    in_=uN_sb[:, (h * NBLK + b) * DIM:(h * NBLK + b + 1) * DIM],
                    identity=ID_sb[:])
                nc.scalar.copy(
                    out=uT_sb[:, h * NSP + b * TILE: h * NSP + (b + 1) * TILE],
                    in_=tp_ps[:])
        for b in range(NSP // CHUNK):
            out_ps = ppv.tile([DIM, CHUNK], f32, tag="vp")
            for h in range(HEADS):
                nc.tensor.matmul(
                    out=out_ps[:],
                    lhsT=P4_sb[:, h * DIM:(h + 1) * DIM],
                    rhs=uT_sb[:, h * NSP + b * CHUNK: h * NSP + (b + 1) * CHUNK],
                    start=(h == 0), stop=(h == HEADS - 1))
            osb = xp.tile([DIM, CHUNK], f32, tag="osb")
            nc.scalar.activation(out=osb[:], in_=out_ps[:],
                                 func=Ident, bias=bo_sb[:, 0:1])
            dmac(outT_d[:, b * CHUNK:(b + 1) * CHUNK], osb[:])

    nc.compile()
    return nc


# ---------------------------------------------------------------------------
# Entry point
# ---------------------------------------------------------------------------

def _prepare(inputs):
    q_nodes = np.asarray(inputs["q_nodes"], np.float32)
    k_edges = np.asarray(inputs["k_edges"], np.float32)
    v_edges = np.asarray(inputs["v_edges"], np.float32)
    Wq = np.asarray(inputs["Wq"], np.float32)
    bq = np.asarray(inputs["bq"], np.float32)
    Wk = np.asarray(inputs["Wk"], np.float32)
    Wv = np.asarray(inputs["Wv"], np.float32)
    bv = np.asarray(inputs["bv"], np.float32)
    Wo = np.asarray(inputs["Wo"], np.float32)
    bo = np.asarray(inputs["bo"], np.float32)
    dst = np.asarray(inputs["edge_index"])[0].astype(np.int64)

    plan = _make_plan(dst)

    eorder = np.argsort(dst, kind="stable")
    starts = np.zeros(N + 1, np.int64)
    np.cumsum(np.bincount(dst, minlength=N), out=starts[1:])
    edges_of = [eorder[starts[n]: starts[n + 1]] for n in range(N)]

    # M_h = B_h @ qT + c_h with B_h = WkS diag(Hm_h) Wq^T; P_h = Wv_h Wo_h
    WkS = Wk * SCALE
    Hm = (np.arange(DIM)[:, None] // DH == np.arange(HEADS)[None, :])
    BT4 = np.empty((DIM, HEADS * DIM), np.float32)
    c4 = np.empty((DIM, HEADS), np.float32)
    P4 = np.empty((DIM, HEADS * DIM), np.float32)
    for h in range(HEADS):
        m = Hm[:, h].astype(np.float32)
        BT4[:, h * DIM:(h + 1) * DIM] = (Wq * m[None, :]) @ WkS.T
        c4[:, h] = WkS @ (m * bq)
        hb = slice(h * DH, (h + 1) * DH)
        P4[:, h * DIM:(h + 1) * DIM] = Wv[:, hb] @ Wo[hb, :]

    consts = {
        "BT4": BT4.astype(np.float16),
        "c4": c4,
        "P4": P4,
        "ID": np.eye(DIM, dtype=np.float32),
        # sum(attn)==1 folds bv through Wo: out = agg@Wo + (bv@Wo + bo)
        "bo": (bv @ Wo + bo).reshape(DIM, 1).astype(np.float32),
    }
    return plan, dst, edges_of, consts, q_nodes, k_edges, v_edges, bo


def kernel(**inputs):
    from concourse.bass_utils import run_bass_kernel_spmd

    (plan, dst, edges_of, consts, q_nodes, k_edges, v_edges, bo) = _prepare(inputs)

    nc = _build_module(plan)

    in_maps = []
    slot_maps = []
    for c in range(NCORES):
        kvs, qT, qslot = _pack_core_inputs(plan, c, k_edges, v_edges,
                                           q_nodes, edges_of)
        m = {"kvs": kvs, "qT": qT}
        m.update(consts)
        in_maps.append(m)
        slot_maps.append(qslot)

    res = run_bass_kernel_spmd(nc, in_maps, core_ids=list(range(NCORES)))
    global LAST_RESULTS
    LAST_RESULTS = res

    out = np.zeros((N, DIM), np.float32)
    for c in range(NCORES):
        outT = res.results[c]["outT"]          # [DIM, nsp]
        qslot = slot_maps[c]
        valid = qslot >= 0
        out[qslot[valid]] = outT[:, : plan.nslot].T[valid]
    deg0 = plan.deg == 0
    if deg0.any():
        out[deg0] = bo
    return out


# revision 21
# speedup vs baseline: 1.8263x; 1.1529x over previous
"""Trainium2 Bass kernel for nn_NodeEdgeCrossAttention.

Strategy (dst-sharded, zero-collective, whole-tile matmuls):
  - Host sorts edges by destination node, assigns nodes to 8 cores round-
    robin by sorted degree rank so all cores share one slot pattern (SPMD
    requires one program), and packs each node's edge run (no padding)
    into 512-col chunks; runs may cross the 128-row tile boundaries.
  - Per chunk, only 8 PE instructions, all with dense 128-wide operands:
      4 score matmuls  ex[e,(h,slot)] = k_tile^T @ M_cols   (one per tile)
      4 U matmuls      U[(h,slot), d|1] = exM_tile^T @ [v_edge-major|ones]
    exp runs on ACT over the full score tile; DVE masks it by the one-hot
    S to give exM = attn-weighted one-hot (wrong-slot entries zeroed), so
    the U matmul both aggregates raw v per (head, slot) and accumulates
    the softmax denominator through the trailing ones column.
  - M folds Wq/Wk/bq per head: M_h = B_h @ qT + c_h (host-precomputed
    B_h, c_h).  bk cancels by softmax shift invariance.  Wv never touches
    edge data: out = sum_h (U_h/den_h) @ P_h + bo', with P_h = Wv[:,h]Wo[h,:]
    and bo' = bv@Wo + bo (sum(attn)==1).
  - Park groups of 3 chunks accumulate U in PSUM, drain via a DRAM scratch
    keyed [head][slot] so the final normalize/transpose/project runs per
    head with plain affine access patterns.
  - Numerics: fp16 k/q/M, bf16 v/exp tensors, fp32 accumulation.
"""

import numpy as np

N, E, DIM, HEADS = 10000, 640000, 128, 4
DH = DIM // HEADS
NCORES = 8
CHUNK = 512
TILE = 128
TPC = CHUNK // TILE
SCALE = DH ** -0.5
VW = TILE + 1          # 129: v-block width per tile (v | ones)
GPC = 3                # chunks per PSUM park group
GEXP = 2               # chunks per exp activation
CAP = 12               # max items per chunk (bounds SL)


class Plan:
    pass


def _make_plan(dst):
    """Pack nodes into a shared chunk/slot layout; no edge padding."""
    deg = np.bincount(dst, minlength=N)
    if deg.max() > CHUNK:
        raise NotImplementedError(f"max degree {deg.max()} > {CHUNK}")
    live = np.nonzero(deg > 0)[0]
    Rl = deg[live].astype(np.int64)

    # Round-robin by sorted rank: core c takes sorted[8i+c]; shared pattern
    # pat[i] = max degree in rank octet i = degree of sorted[8i].
    order = np.argsort(-Rl, kind="stable")
    core_nodes = [[int(live[n]) for n in order[c::NCORES]]
                  for c in range(NCORES)]
    pat = Rl[order[::NCORES]]
    L = len(pat)

    # First-fit-decreasing into 512-col chunks (item count capped).
    cks = []
    space = []
    for pi in range(L):
        R = int(pat[pi])
        for b in range(len(cks)):
            if space[b] >= R and len(cks[b]) < CAP:
                cks[b].append((pi, CHUNK - space[b], R))
                space[b] -= R
                break
        else:
            cks.append([(pi, 0, R)])
            space.append(CHUNK - R)

    nchunks = len(cks)
    chunks = []
    for ci in range(nchunks):
        items = []
        pieces = []   # host-side S construction only
        for j, (pi, start, R) in enumerate(cks[ci]):
            items.append({"pi": pi, "j": j, "start": start, "R": R})
            col, rem, ioff = start, R, 0
            while rem > 0:
                t, c0 = col // TILE, col % TILE
                r = min(rem, TILE - c0)
                pieces.append({"j": j, "tile": t, "col0": c0, "r": r,
                               "ioff": ioff})
                col += r; rem -= r; ioff += r
        chunks.append({"items": items, "pieces": pieces})
    SL = max(len(ch["items"]) for ch in chunks)

    p = Plan()
    p.sl = SL
    p.kvw = CHUNK + TPC * VW + TPC * SL
    p.deg = deg
    p.core_nodes = core_nodes
    p.chunks = chunks
    p.nchunks = nchunks
    p.cols = nchunks * CHUNK
    p.nslot = nchunks * SL
    p.nslot_b = ((p.nslot + TILE - 1) // TILE) * TILE
    p.nsp = ((p.nslot + CHUNK - 1) // CHUNK) * CHUNK
    return p


def _pack_core_inputs(plan, c, k_edges, v_edges, q_nodes, edges_of):
    """Per-core fused kvs [128, nchunks*KVW] f16, qT [128, nsp] f16, qslot."""
    import ml_dtypes
    bf16 = ml_dtypes.bfloat16
    cols = plan.cols
    edge_order = np.full(cols, -1, np.int64)
    qslot = np.full(plan.nslot, -1, np.int64)
    cn = plan.core_nodes[c]
    SL = plan.sl
    node_of = []
    for ci, ch in enumerate(plan.chunks):
        nmap = {}
        for it in ch["items"]:
            if it["pi"] < 0 or it["pi"] >= len(cn):
                continue
            node = cn[it["pi"]]
            nmap[it["j"]] = node
            d = plan.deg[node]
            g0 = ci * CHUNK + it["start"]
            edge_order[g0: g0 + d] = edges_of[node]
            qslot[ci * SL + it["j"]] = node
        node_of.append(nmap)

    valid = edge_order >= 0
    idx = np.where(valid, edge_order, 0)
    kT = np.where(valid[:, None], k_edges[idx], 0.0).astype(np.float16).T
    vE = np.where(valid[:, None], v_edges[idx], 0.0).astype(bf16)

    # edge-major v blocks with a trailing ones column per tile
    vem = np.zeros((TILE, plan.nchunks, TPC, VW), bf16)
    vem[:, :, :, 0:TILE] = vE.reshape(plan.nchunks, TPC, TILE, DIM
                                      ).transpose(2, 0, 1, 3)
    vem[:, :, :, TILE] = bf16(1.0)

    # one-hot S: [128, nchunks*TPC*SL], col (chunk, tile, slot_j)
    S = np.zeros((TILE, plan.nchunks * TPC * SL), np.float32)
    for ci, ch in enumerate(plan.chunks):
        nmap = node_of[ci]
        for pc in ch["pieces"]:
            node = nmap.get(pc["j"], -1)
            if node < 0:
                continue
            d = int(plan.deg[node])
            cov = min(d - pc["ioff"], pc["r"])
            if cov <= 0:
                continue
            col = (ci * TPC + pc["tile"]) * SL + pc["j"]
            S[pc["col0"]: pc["col0"] + cov, col] = 1.0
    Sbits = S.astype(bf16).view(np.float16)

    kvs = np.empty((TILE, plan.nchunks * plan.kvw), np.float16)
    kvw = kvs.reshape(TILE, plan.nchunks, plan.kvw)
    kvw[:, :, 0:CHUNK] = kT.reshape(TILE, plan.nchunks, CHUNK)
    kvw[:, :, CHUNK:CHUNK + TPC * VW] = vem.reshape(
        TILE, plan.nchunks, TPC * VW).view(np.float16)
    kvw[:, :, CHUNK + TPC * VW:] = Sbits.reshape(TILE, plan.nchunks, TPC * SL)

    qvalid = qslot >= 0
    qidx = np.where(qvalid, qslot, 0)
    qT = np.zeros((DIM, plan.nsp), np.float16)
    qT[:, : plan.nslot] = np.where(qvalid[:, None], q_nodes[qidx], 0.0
                                   ).astype(np.float16).T
    return kvs, qT, qslot


# ---------------------------------------------------------------------------
# Device kernel emission
# ---------------------------------------------------------------------------

def _build_module(plan):
    import concourse.bacc as bacc
    import concourse.mybir as mybir
    import concourse.tile as tile
    from contextlib import ExitStack

    f16 = mybir.dt.float16
    bf = mybir.dt.bfloat16
    f32 = mybir.dt.float32
    NSP = plan.nsp
    NBLK = plan.nslot_b // TILE
    SL = plan.sl
    SW = TPC * SL          # score cols per chunk... per tile: 4*SL? no: HEADS*SL
    HS = HEADS * SL        # score cols per tile (h-major: h*SL+s)
    KVW = plan.kvw
    NCH = plan.nchunks
    UW = VW                # 129 park row width per chunk (U | den)

    nc = bacc.Bacc("TRN2", debug=False, num_devices=NCORES)

    kvs_d = nc.dram_tensor("kvs", [TILE, NCH * KVW], f16,
                           kind="ExternalInput")
    qT_d = nc.dram_tensor("qT", [DIM, NSP], f16, kind="ExternalInput")
    BT4_d = nc.dram_tensor("BT4", [DIM, HEADS * DIM], f16, kind="ExternalInput")
    c4_d = nc.dram_tensor("c4", [DIM, HEADS], f32, kind="ExternalInput")
    P4_d = nc.dram_tensor("P4", [DIM, HEADS * DIM], f32, kind="ExternalInput")
    ID_d = nc.dram_tensor("ID", [DIM, DIM], f32, kind="ExternalInput")
    bo_d = nc.dram_tensor("bo", [DIM, 1], f32, kind="ExternalInput")
    accD = nc.dram_tensor("accD", [HEADS, plan.nslot, UW], f32, kind="Internal")
    outT_d = nc.dram_tensor("outT", [DIM, NSP], f32, kind="ExternalOutput")

    Exp = mybir.ActivationFunctionType.Exp
    Ident = mybir.ActivationFunctionType.Identity
    mult = mybir.AluOpType.mult
    amax = mybir.AluOpType.max

    with ExitStack() as ctx:
        tc = ctx.enter_context(tile.TileContext(nc))
        cp = ctx.enter_context(tc.tile_pool(name="const", bufs=1))
        sp = ctx.enter_context(tc.tile_pool(name="persist", bufs=1))
        iop = ctx.enter_context(tc.tile_pool(name="io", bufs=6))
        xp = ctx.enter_context(tc.tile_pool(name="work", bufs=3))
        pp = ctx.enter_context(tc.tile_pool(name="ps", bufs=2, space="PSUM"))
        ppv = ctx.enter_context(tc.tile_pool(name="psv", bufs=3, space="PSUM"))

        def dmac(tile_ap, dram_ap):
            nc.sync.dma_start(out=tile_ap, in_=dram_ap)

        BT4_sb = cp.tile([DIM, HEADS * DIM], f16); dmac(BT4_sb[:], BT4_d[:, :])
        c4_sb = cp.tile([DIM, HEADS], f32); dmac(c4_sb[:], c4_d[:, :])
        P4_sb = cp.tile([DIM, HEADS * DIM], f32); dmac(P4_sb[:], P4_d[:, :])
        ID_sb = cp.tile([DIM, DIM], f32); dmac(ID_sb[:], ID_d[:, :])
        bo_sb = cp.tile([DIM, 1], f32); dmac(bo_sb[:], bo_d[:, :])
        qT_sb = sp.tile([DIM, NSP], f16); dmac(qT_sb[:], qT_d[:, :])

        # M4_sb per chunk block: col ci*HS + h*SL + j  (score-all rhs order)
        M4_sb = sp.tile([DIM, NCH * HS], f16)
        M4r = M4_sb[:].rearrange("p (c h s) -> p c h s", h=HEADS, s=SL)

        # ---- Stage A: M = B_h @ qT + c_h over chunk-aligned blocks ----
        CG = CHUNK // SL                 # chunks per stage-A block
        for cb in range(0, NCH, CG):
            ce = min(cb + CG, NCH)
            w = (ce - cb) * SL
            for h in range(HEADS):
                ps = ppv.tile([DIM, CHUNK], f32, tag="vp")
                nc.tensor.matmul(out=ps[:, 0:w],
                                 lhsT=BT4_sb[:, h * DIM:(h + 1) * DIM],
                                 rhs=qT_sb[:, cb * SL: cb * SL + w],
                                 start=True, stop=True)
                nc.scalar.activation(
                    out=M4r[:, cb:ce, h, :],
                    in_=ps[:, 0:w].rearrange("p (c s) -> p c s", s=SL),
                    func=Ident, bias=c4_sb[:, h:h + 1])

        # ---- Steady state ----
        park = None
        ex_ps = None
        group = []
        for ci, ch in enumerate(plan.chunks):
            kvt = iop.tile([TILE, KVW], f16, tag="kv")
            # alternate the two HWDGE rings so chunk loads overlap (each
            # ring drains its DMAs in FIFO order)
            eng = nc.sync if ci % 2 == 0 else nc.scalar
            eng.dma_start(out=kvt[:], in_=kvs_d[:, ci * KVW:(ci + 1) * KVW])
            kc = kvt[:, 0:CHUNK]

            gi = ci % GEXP
            if gi == 0:
                ex_ps = pp.tile([TILE, GEXP * TPC * HS], f32, tag="ex")
            for t in range(TPC):
                nc.tensor.matmul(
                    out=ex_ps[:, (gi * TPC + t) * HS:(gi * TPC + t + 1) * HS],
                    lhsT=kc[:, t * TILE:(t + 1) * TILE],
                    rhs=M4_sb[:, ci * HS:(ci + 1) * HS],
                    start=True, stop=True)
            group.append((ci, kvt, gi))
            if gi < GEXP - 1 and ci < plan.nchunks - 1:
                continue

            exE_sb = xp.tile([TILE, GEXP * TPC * HS], bf, tag="exE")
            nc.scalar.activation(out=exE_sb[:], in_=ex_ps[:], func=Exp)

            for (cj, kvtj, gj) in group:
                Sc = kvtj[:, CHUNK + TPC * VW:KVW].bitcast(bf)
                vem = kvtj[:, CHUNK:CHUNK + TPC * VW].bitcast(bf)
                exM_sb = xp.tile([TILE, TPC * HS], bf, tag="exM")
                nc.vector.tensor_tensor(
                    out=exM_sb[:].rearrange("p (t h s) -> p t h s",
                                            t=TPC, h=HEADS),
                    in0=exE_sb[:, gj * TPC * HS:(gj + 1) * TPC * HS]
                        .rearrange("p (t h s) -> p t h s", t=TPC, h=HEADS),
                    in1=Sc[:].rearrange("p (t s) -> p t s", t=TPC)
                        .unsqueeze(2).to_broadcast([TILE, TPC, HEADS, SL]),
                    op=mult)

                g2 = cj % GPC
                if g2 == 0:
                    park = pp.tile([HS, GPC * UW], f32, tag="park")
                for t in range(TPC):
                    nc.tensor.matmul(
                        out=park[:, g2 * UW:(g2 + 1) * UW],
                        lhsT=exM_sb[:, t * HS:(t + 1) * HS],
                        rhs=vem[:, t * VW:(t + 1) * VW],
                        start=(t == 0), stop=(t == TPC - 1))
                if g2 == GPC - 1 or cj == plan.nchunks - 1:
                    g0 = (cj // GPC) * GPC
                    used = cj - g0 + 1
                    stage = xp.tile([HS, GPC * UW], f32, tag="stage")
                    nc.scalar.copy(out=stage[:, 0:used * UW],
                                   in_=park[:, 0:used * UW])
                    for h in range(HEADS):
                        nc.gpsimd.dma_start(
                            out=accD[h, g0 * SL:(cj + 1) * SL, :]
                                .rearrange("(c j) w -> j c w", j=SL),
                            in_=stage[h * SL:(h + 1) * SL, 0:used * UW]
                                .rearrange("j (c w) -> j c w", w=UW))
            group = []

        # ---- Final: per head, read back, normalize, transpose, project ----
        accR = sp.tile([TILE, HEADS * NBLK * UW], f32)
        nc.gpsimd.memset(accR[:], 0.0)
        full = plan.nslot // TILE
        tail = plan.nslot - full * TILE
        rden_sb = sp.tile([TILE, HEADS * NBLK], f32)
        uN_sb = sp.tile([TILE, HEADS * NBLK * DIM], f32)
        uT_sb = sp.tile([TILE, HEADS * NSP], f32)
        nc.gpsimd.memset(uT_sb[:], 0.0)
        for h in range(HEADS):
            base = h * NBLK * UW
            if full:
                dmac(accR[:, base:base + full * UW]
                     .rearrange("p (b w) -> p b w", w=UW),
                     accD[h, 0:full * TILE, :]
                     .rearrange("(b p) w -> p b w", p=TILE))
            if tail:
                dmac(accR[0:tail, base + full * UW:base + (full + 1) * UW],
                     accD[h, full * TILE:plan.nslot, :])
            accRh = accR[:, base:base + NBLK * UW].rearrange(
                "p (b w) -> p b w", w=UW)
            rdenh = rden_sb[:, h * NBLK:(h + 1) * NBLK]
            nc.vector.tensor_scalar(
                out=rdenh.rearrange("p b -> p b").unsqueeze(-1),
                in0=accRh[:, :, TILE:TILE + 1],
                scalar1=1e-30, scalar2=None, op0=amax)
            nc.vector.reciprocal(out=rdenh, in_=rdenh)
            nc.vector.tensor_tensor(
                out=uN_sb[:, h * NBLK * DIM:(h + 1) * NBLK * DIM]
                    .rearrange("p (b d) -> p b d", d=DIM),
                in0=accRh[:, :, 0:TILE],
                in1=rdenh.rearrange("p b -> p b").unsqueeze(-1)
                    .to_broadcast([TILE, NBLK, DIM]),
                op=mult)
            for b in range(NBLK):
                tp_ps = ppv.tile([DIM, TILE], f32, tag="vp")
                nc.tensor.transpose(
                    out=tp_ps[:],
                    in_=uN_sb[:, (h * NBLK + b) * DIM:(h * NBLK + b + 1) * DIM],
                    identity=ID_sb[:])
                nc.scalar.copy(
                    out=uT_sb[:, h * NSP + b * TILE: h * NSP + (b + 1) * TILE],
                    in_=tp_ps[:])
        for b in range(NSP // CHUNK):
            out_ps = ppv.tile([DIM, CHUNK], f32, tag="vp")
            for h in range(HEADS):
                nc.tensor.matmul(
                    out=out_ps[:],
                    lhsT=P4_sb[:, h * DIM:(h + 1) * DIM],
                    rhs=uT_sb[:, h * NSP + b * CHUNK: h * NSP + (b + 1) * CHUNK],
                    start=(h == 0), stop=(h == HEADS - 1))
            osb = xp.tile([DIM, CHUNK], f32, tag="osb")
            nc.scalar.activation(out=osb[:], in_=out_ps[:],
                                 func=Ident, bias=bo_sb[:, 0:1])
            dmac(outT_d[:, b * CHUNK:(b + 1) * CHUNK], osb[:])

    nc.compile()
    return nc


# ---------------------------------------------------------------------------
# Entry point
# ---------------------------------------------------------------------------

def _prepare(inputs):
    q_nodes = np.asarray(inputs["q_nodes"], np.float32)
    k_edges = np.asarray(inputs["k_edges"], np.float32)
    v_edges = np.asarray(inputs["v_edges"], np.float32)
    Wq = np.asarray(inputs["Wq"], np.float32)
    bq = np.asarray(inputs["bq"], np.float32)
    Wk = np.asarray(inputs["Wk"], np.float32)
    Wv = np.asarray(inputs["Wv"], np.float32)
    bv = np.asarray(inputs["bv"], np.float32)
    Wo = np.asarray(inputs["Wo"], np.float32)
    bo = np.asarray(inputs["bo"], np.float32)
    dst = np.asarray(inputs["edge_index"])[0].astype(np.int64)

    plan = _make_plan(dst)

    eorder = np.argsort(dst, kind="stable")
    starts = np.zeros(N + 1, np.int64)
    np.cumsum(np.bincount(dst, minlength=N), out=starts[1:])
    edges_of = [eorder[starts[n]: starts[n + 1]] for n in range(N)]

    # M_h = B_h @ qT + c_h with B_h = WkS diag(Hm_h) Wq^T; P_h = Wv_h Wo_h
    WkS = Wk * SCALE
    Hm = (np.arange(DIM)[:, None] // DH == np.arange(HEADS)[None, :])
    BT4 = np.empty((DIM, HEADS * DIM), np.float32)
    c4 = np.empty((DIM, HEADS), np.float32)
    P4 = np.empty((DIM, HEADS * DIM), np.float32)
    for h in range(HEADS):
        m = Hm[:, h].astype(np.float32)
        BT4[:, h * DIM:(h + 1) * DIM] = (Wq * m[None, :]) @ WkS.T
        c4[:, h] = WkS @ (m * bq)
        hb = slice(h * DH, (h + 1) * DH)
        P4[:, h * DIM:(h + 1) * DIM] = Wv[:, hb] @ Wo[hb, :]

    consts = {
        "BT4": BT4.astype(np.float16),
        "c4": c4,
        "P4": P4,
        "ID": np.eye(DIM, dtype=np.float32),
        # sum(attn)==1 folds bv through Wo: out = agg@Wo + (bv@Wo + bo)
        "bo": (bv @ Wo + bo).reshape(DIM, 1).astype(np.float32),
    }
    return plan, dst, edges_of, consts, q_nodes, k_edges, v_edges, bo


def kernel(**inputs):
    from concourse.bass_utils import run_bass_kernel_spmd

    (plan, dst, edges_of, consts, q_nodes, k_edges, v_edges, bo) = _prepare(inputs)

    nc = _build_module(plan)

    in_maps = []
    slot_maps = []
    for c in range(NCORES):
        kvs, qT, qslot = _pack_core_inputs(plan, c, k_edges, v_edges,
                                           q_nodes, edges_of)
        m = {"kvs": kvs, "qT": qT}
        m.update(consts)
        in_maps.append(m)
        slot_maps.append(qslot)

    res = run_bass_kernel_spmd(nc, in_maps, core_ids=list(range(NCORES)))
    global LAST_RESULTS
    LAST_RESULTS = res

    out = np.zeros((N, DIM), np.float32)
    for c in range(NCORES):
        outT = res.results[c]["outT"]          # [DIM, nsp]
        qslot = slot_maps[c]
        valid = qslot >= 0
        out[qslot[valid]] = outT[:, : plan.nslot].T[valid]
    deg0 = plan.deg == 0
    if deg0.any():
        out[deg0] = bo
    return out


# revision 23
# speedup vs baseline: 1.8597x; 1.0183x over previous
"""Trainium2 Bass kernel for nn_NodeEdgeCrossAttention.

Strategy (dst-sharded, zero-collective, whole-tile matmuls):
  - Host sorts edges by destination node, assigns nodes to 8 cores round-
    robin by sorted degree rank so all cores share one slot pattern (SPMD
    requires one program), and packs each node's edge run (no padding)
    into 512-col chunks; runs may cross the 128-row tile boundaries.
  - Per chunk, only 8 PE instructions, all with dense 128-wide operands:
      4 score matmuls  ex[e,(h,slot)] = k_tile^T @ M_cols   (one per tile)
      4 U matmuls      U[(h,slot), d|1] = exM_tile^T @ [v_edge-major|ones]
    exp runs on ACT over the full score tile; DVE masks it by the one-hot
    S to give exM = attn-weighted one-hot (wrong-slot entries zeroed), so
    the U matmul both aggregates raw v per (head, slot) and accumulates
    the softmax denominator through the trailing ones column.
  - M folds Wq/Wk/bq per head: M_h = B_h @ qT + c_h (host-precomputed
    B_h, c_h).  bk cancels by softmax shift invariance.  Wv never touches
    edge data: out = sum_h (U_h/den_h) @ P_h + bo', with P_h = Wv[:,h]Wo[h,:]
    and bo' = bv@Wo + bo (sum(attn)==1).
  - Park groups of 3 chunks accumulate U in PSUM, drain via a DRAM scratch
    keyed [head][slot] so the final normalize/transpose/project runs per
    head with plain affine access patterns.
  - Numerics: fp16 k/q/M, bf16 v/exp tensors, fp32 accumulation.
"""

import numpy as np

N, E, DIM, HEADS = 10000, 640000, 128, 4
DH = DIM // HEADS
NCORES = 8
CHUNK = 512
TILE = 128
TPC = CHUNK // TILE
SCALE = DH ** -0.5
VW = TILE + 1          # 129: v-block width per tile (v | ones)
GPC = 3                # chunks per PSUM park group
GEXP = 2               # chunks per exp activation
CAP = 12               # max items per chunk (bounds SL)


class Plan:
    pass


def _make_plan(dst):
    """Pack nodes into a shared chunk/slot layout; no edge padding."""
    deg = np.bincount(dst, minlength=N)
    if deg.max() > CHUNK:
        raise NotImplementedError(f"max degree {deg.max()} > {CHUNK}")
    live = np.nonzero(deg > 0)[0]
    Rl = deg[live].astype(np.int64)

    # Round-robin by sorted rank: core c takes sorted[8i+c]; shared pattern
    # pat[i] = max degree in rank octet i = degree of sorted[8i].
    order = np.argsort(-Rl, kind="stable")
    core_nodes = [[int(live[n]) for n in order[c::NCORES]]
                  for c in range(NCORES)]
    pat = Rl[order[::NCORES]]
    L = len(pat)

    # First-fit-decreasing into 512-col chunks (item count capped).
    cks = []
    space = []
    for pi in range(L):
        R = int(pat[pi])
        for b in range(len(cks)):
            if space[b] >= R and len(cks[b]) < CAP:
                cks[b].append((pi, CHUNK - space[b], R))
                space[b] -= R
                break
        else:
            cks.append([(pi, 0, R)])
            space.append(CHUNK - R)

    nchunks = len(cks)
    chunks = []
    for ci in range(nchunks):
        items = []
        pieces = []   # host-side S construction only
        for j, (pi, start, R) in enumerate(cks[ci]):
            items.append({"pi": pi, "j": j, "start": start, "R": R})
            col, rem, ioff = start, R, 0
            while rem > 0:
                t, c0 = col // TILE, col % TILE
                r = min(rem, TILE - c0)
                pieces.append({"j": j, "tile": t, "col0": c0, "r": r,
                               "ioff": ioff})
                col += r; rem -= r; ioff += r
        chunks.append({"items": items, "pieces": pieces})
    SL = max(len(ch["items"]) for ch in chunks)

    p = Plan()
    p.sl = SL
    p.kvw = CHUNK + TPC * VW + TPC * SL
    p.deg = deg
    p.core_nodes = core_nodes
    p.chunks = chunks
    p.nchunks = nchunks
    p.cols = nchunks * CHUNK
    p.nslot = nchunks * SL
    p.nslot_b = ((p.nslot + TILE - 1) // TILE) * TILE
    p.nsp = ((p.nslot + CHUNK - 1) // CHUNK) * CHUNK
    return p


def _pack_core_inputs(plan, c, k_edges, v_edges, q_nodes, edges_of):
    """Per-core fused kvs [128, nchunks*KVW] f16, qT [128, nsp] f16, qslot."""
    import ml_dtypes
    bf16 = ml_dtypes.bfloat16
    cols = plan.cols
    edge_order = np.full(cols, -1, np.int64)
    qslot = np.full(plan.nslot, -1, np.int64)
    cn = plan.core_nodes[c]
    SL = plan.sl
    node_of = []
    for ci, ch in enumerate(plan.chunks):
        nmap = {}
        for it in ch["items"]:
            if it["pi"] < 0 or it["pi"] >= len(cn):
                continue
            node = cn[it["pi"]]
            nmap[it["j"]] = node
            d = plan.deg[node]
            g0 = ci * CHUNK + it["start"]
            edge_order[g0: g0 + d] = edges_of[node]
            qslot[ci * SL + it["j"]] = node
        node_of.append(nmap)

    valid = edge_order >= 0
    idx = np.where(valid, edge_order, 0)
    kT = np.where(valid[:, None], k_edges[idx], 0.0).astype(np.float16).T
    vE = np.where(valid[:, None], v_edges[idx], 0.0).astype(bf16)

    # edge-major v blocks with a trailing ones column per tile
    vem = np.zeros((TILE, plan.nchunks, TPC, VW), bf16)
    vem[:, :, :, 0:TILE] = vE.reshape(plan.nchunks, TPC, TILE, DIM
                                      ).transpose(2, 0, 1, 3)
    vem[:, :, :, TILE] = bf16(1.0)

    # one-hot S: [128, nchunks*TPC*SL], col (chunk, tile, slot_j)
    S = np.zeros((TILE, plan.nchunks * TPC * SL), np.float32)
    for ci, ch in enumerate(plan.chunks):
        nmap = node_of[ci]
        for pc in ch["pieces"]:
            node = nmap.get(pc["j"], -1)
            if node < 0:
                continue
            d = int(plan.deg[node])
            cov = min(d - pc["ioff"], pc["r"])
            if cov <= 0:
                continue
            col = (ci * TPC + pc["tile"]) * SL + pc["j"]
            S[pc["col0"]: pc["col0"] + cov, col] = 1.0
    Sbits = S.astype(bf16).view(np.float16)

    kvs = np.empty((TILE, plan.nchunks * plan.kvw), np.float16)
    kvw = kvs.reshape(TILE, plan.nchunks, plan.kvw)
    kvw[:, :, 0:CHUNK] = kT.reshape(TILE, plan.nchunks, CHUNK)
    kvw[:, :, CHUNK:CHUNK + TPC * VW] = vem.reshape(
        TILE, plan.nchunks, TPC * VW).view(np.float16)
    kvw[:, :, CHUNK + TPC * VW:] = Sbits.reshape(TILE, plan.nchunks, TPC * SL)

    qvalid = qslot >= 0
    qidx = np.where(qvalid, qslot, 0)
    qT = np.zeros((DIM, plan.nsp), np.float16)
    qT[:, : plan.nslot] = np.where(qvalid[:, None], q_nodes[qidx], 0.0
                                   ).astype(np.float16).T
    return kvs, qT, qslot


# ---------------------------------------------------------------------------
# Device kernel emission
# ---------------------------------------------------------------------------

def _build_module(plan):
    import concourse.bacc as bacc
    import concourse.mybir as mybir
    import concourse.tile as tile
    from contextlib import ExitStack

    f16 = mybir.dt.float16
    bf = mybir.dt.bfloat16
    f32 = mybir.dt.float32
    NSP = plan.nsp
    NBLK = plan.nslot_b // TILE
    SL = plan.sl
    SW = TPC * SL          # score cols per chunk... per tile: 4*SL? no: HEADS*SL
    HS = HEADS * SL        # score cols per tile (h-major: h*SL+s)
    KVW = plan.kvw
    NCH = plan.nchunks
    UW = VW                # 129 park row width per chunk (U | den)

    nc = bacc.Bacc("TRN2", debug=False, num_devices=NCORES)

    kvs_d = nc.dram_tensor("kvs", [TILE, NCH * KVW], f16,
                           kind="ExternalInput")
    qT_d = nc.dram_tensor("qT", [DIM, NSP], f16, kind="ExternalInput")
    BT4_d = nc.dram_tensor("BT4", [DIM, HEADS * DIM], f16, kind="ExternalInput")
    c4_d = nc.dram_tensor("c4", [DIM, HEADS], f32, kind="ExternalInput")
    P4_d = nc.dram_tensor("P4", [DIM, HEADS * DIM], f32, kind="ExternalInput")
    ID_d = nc.dram_tensor("ID", [DIM, DIM], f32, kind="ExternalInput")
    bo_d = nc.dram_tensor("bo", [DIM, 1], f32, kind="ExternalInput")
    accD = nc.dram_tensor("accD", [HEADS, plan.nslot, UW], f32, kind="Internal")
    outT_d = nc.dram_tensor("outT", [DIM, NSP], f32, kind="ExternalOutput")

    Exp = mybir.ActivationFunctionType.Exp
    Ident = mybir.ActivationFunctionType.Identity
    mult = mybir.AluOpType.mult
    amax = mybir.AluOpType.max

    with ExitStack() as ctx:
        tc = ctx.enter_context(tile.TileContext(nc))
        cp = ctx.enter_context(tc.tile_pool(name="const", bufs=1))
        sp = ctx.enter_context(tc.tile_pool(name="persist", bufs=1))
        iop = ctx.enter_context(tc.tile_pool(name="io", bufs=9))
        xp = ctx.enter_context(tc.tile_pool(name="work", bufs=3))
        pp = ctx.enter_context(tc.tile_pool(name="ps", bufs=3, space="PSUM"))
        ppk = ctx.enter_context(tc.tile_pool(name="psk", bufs=2, space="PSUM"))
        ppv = ctx.enter_context(tc.tile_pool(name="psv", bufs=3, space="PSUM"))

        def dmac(tile_ap, dram_ap):
            nc.sync.dma_start(out=tile_ap, in_=dram_ap)

        BT4_sb = cp.tile([DIM, HEADS * DIM], f16); dmac(BT4_sb[:], BT4_d[:, :])
        c4_sb = cp.tile([DIM, HEADS], f32); dmac(c4_sb[:], c4_d[:, :])
        P4_sb = cp.tile([DIM, HEADS * DIM], f32); dmac(P4_sb[:], P4_d[:, :])
        ID_sb = cp.tile([DIM, DIM], f32); dmac(ID_sb[:], ID_d[:, :])
        bo_sb = cp.tile([DIM, 1], f32); dmac(bo_sb[:], bo_d[:, :])
        qT_sb = sp.tile([DIM, NSP], f16); dmac(qT_sb[:], qT_d[:, :])

        # M4_sb per chunk block: col ci*HS + h*SL + j  (score-all rhs order)
        M4_sb = sp.tile([DIM, NCH * HS], f16)
        M4r = M4_sb[:].rearrange("p (c h s) -> p c h s", h=HEADS, s=SL)

        # ---- Stage A: M = B_h @ qT + c_h over chunk-aligned blocks ----
        CG = CHUNK // SL                 # chunks per stage-A block
        for cb in range(0, NCH, CG):
            ce = min(cb + CG, NCH)
            w = (ce - cb) * SL
            for h in range(HEADS):
                ps = ppv.tile([DIM, CHUNK], f32, tag="vp")
                nc.tensor.matmul(out=ps[:, 0:w],
                                 lhsT=BT4_sb[:, h * DIM:(h + 1) * DIM],
                                 rhs=qT_sb[:, cb * SL: cb * SL + w],
                                 start=True, stop=True)
                nc.scalar.activation(
                    out=M4r[:, cb:ce, h, :],
                    in_=ps[:, 0:w].rearrange("p (c s) -> p c s", s=SL),
                    func=Ident, bias=c4_sb[:, h:h + 1])

        # ---- Steady state ----
        park = None
        ex_ps = None
        group = []
        for ci, ch in enumerate(plan.chunks):
            kvt = iop.tile([TILE, KVW], f16, tag="kv")
            # alternate the two HWDGE rings so chunk loads overlap (each
            # ring drains its DMAs in FIFO order)
            eng = nc.sync if ci % 2 == 0 else nc.scalar
            eng.dma_start(out=kvt[:], in_=kvs_d[:, ci * KVW:(ci + 1) * KVW])
            kc = kvt[:, 0:CHUNK]

            gi = ci % GEXP
            if gi == 0:
                ex_ps = pp.tile([TILE, GEXP * TPC * HS], f32, tag="ex")
            for t in range(TPC):
                nc.tensor.matmul(
                    out=ex_ps[:, (gi * TPC + t) * HS:(gi * TPC + t + 1) * HS],
                    lhsT=kc[:, t * TILE:(t + 1) * TILE],
                    rhs=M4_sb[:, ci * HS:(ci + 1) * HS],
                    start=True, stop=True)
            group.append((ci, kvt, gi))
            if gi < GEXP - 1 and ci < plan.nchunks - 1:
                continue

            exE_sb = xp.tile([TILE, GEXP * TPC * HS], bf, tag="exE")
            nc.scalar.activation(out=exE_sb[:], in_=ex_ps[:], func=Exp)

            for (cj, kvtj, gj) in group:
                Sc = kvtj[:, CHUNK + TPC * VW:KVW].bitcast(bf)
                vem = kvtj[:, CHUNK:CHUNK + TPC * VW].bitcast(bf)
                exM_sb = xp.tile([TILE, TPC * HS], bf, tag="exM")
                nc.vector.tensor_tensor(
                    out=exM_sb[:].rearrange("p (t h s) -> p t h s",
                                            t=TPC, h=HEADS),
                    in0=exE_sb[:, gj * TPC * HS:(gj + 1) * TPC * HS]
                        .rearrange("p (t h s) -> p t h s", t=TPC, h=HEADS),
                    in1=Sc[:].rearrange("p (t s) -> p t s", t=TPC)
                        .unsqueeze(2).to_broadcast([TILE, TPC, HEADS, SL]),
                    op=mult)

                g2 = cj % GPC
                if g2 == 0:
                    park = ppk.tile([HS, GPC * UW], f32, tag="park")
                for t in range(TPC):
                    nc.tensor.matmul(
                        out=park[:, g2 * UW:(g2 + 1) * UW],
                        lhsT=exM_sb[:, t * HS:(t + 1) * HS],
                        rhs=vem[:, t * VW:(t + 1) * VW],
                        start=(t == 0), stop=(t == TPC - 1))
                if g2 == GPC - 1 or cj == plan.nchunks - 1:
                    g0 = (cj // GPC) * GPC
                    used = cj - g0 + 1
                    stage = xp.tile([HS, GPC * UW], f32, tag="stage")
                    nc.scalar.copy(out=stage[:, 0:used * UW],
                                   in_=park[:, 0:used * UW])
                    for h in range(HEADS):
                        nc.gpsimd.dma_start(
                            out=accD[h, g0 * SL:(cj + 1) * SL, :]
                                .rearrange("(c j) w -> j c w", j=SL),
                            in_=stage[h * SL:(h + 1) * SL, 0:used * UW]
                                .rearrange("j (c w) -> j c w", w=UW))
            group = []

        # ---- Final: per head, read back, normalize, transpose, project ----
        accR = sp.tile([TILE, HEADS * NBLK * UW], f32)
        nc.gpsimd.memset(accR[:], 0.0)
        full = plan.nslot // TILE
        tail = plan.nslot - full * TILE
        rden_sb = sp.tile([TILE, HEADS * NBLK], f32)
        uN_sb = sp.tile([TILE, HEADS * NBLK * DIM], f32)
        uT_sb = sp.tile([TILE, HEADS * NSP], f32)
        nc.gpsimd.memset(uT_sb[:], 0.0)
        for h in range(HEADS):
            base = h * NBLK * UW
            if full:
                dmac(accR[:, base:base + full * UW]
                     .rearrange("p (b w) -> p b w", w=UW),
                     accD[h, 0:full * TILE, :]
                     .rearrange("(b p) w -> p b w", p=TILE))
            if tail:
                dmac(accR[0:tail, base + full * UW:base + (full + 1) * UW],
                     accD[h, full * TILE:plan.nslot, :])
            accRh = accR[:, base:base + NBLK * UW].rearrange(
                "p (b w) -> p b w", w=UW)
            rdenh = rden_sb[:, h * NBLK:(h + 1) * NBLK]
            nc.vector.tensor_scalar(
                out=rdenh.rearrange("p b -> p b").unsqueeze(-1),
                in0=accRh[:, :, TILE:TILE + 1],
                scalar1=1e-30, scalar2=None, op0=amax)
            nc.vector.reciprocal(out=rdenh, in_=rdenh)
            nc.vector.tensor_tensor(
                out=uN_sb[:, h * NBLK * DIM:(h + 1) * NBLK * DIM]
                    .rearrange("p (b d) -> p b d", d=DIM),
                in0=accRh[:, :, 0:TILE],
                in1=rdenh.rearrange("p b -> p b").unsqueeze(-1)
                    .to_broadcast([TILE, NBLK, DIM]),
                op=mult)
            for b in range(NBLK):
                tp_ps = ppv.tile([DIM, TILE], f32, tag="vp")
                nc.tensor.transpose(
                    out=tp_ps[:],
                    in_=uN_sb[:, (h * NBLK + b) * DIM:(h * NBLK + b + 1) * DIM],
                    identity=ID_sb[:])
                cpy = nc.scalar.copy if b % 2 == 0 else nc.vector.tensor_copy
                cpy(out=uT_sb[:, h * NSP + b * TILE:
                              h * NSP + (b + 1) * TILE],
                    in_=tp_ps[:])
        for b in range(NSP // CHUNK):
            out_ps = ppv.tile([DIM, CHUNK], f32, tag="vp")
            for h in range(HEADS):
                nc.tensor.matmul(
                    out=out_ps[:],
                    lhsT=P4_sb[:, h * DIM:(h + 1) * DIM],
                    rhs=uT_sb[:, h * NSP + b * CHUNK: h * NSP + (b + 1) * CHUNK],
                    start=(h == 0), stop=(h == HEADS - 1))
            osb = xp.tile([DIM, CHUNK], f32, tag="osb")
            nc.scalar.activation(out=osb[:], in_=out_ps[:],
                                 func=Ident, bias=bo_sb[:, 0:1])
            dmac(outT_d[:, b * CHUNK:(b + 1) * CHUNK], osb[:])

    nc.compile()
    return nc


# ---------------------------------------------------------------------------
# Entry point
# ---------------------------------------------------------------------------

def _prepare(inputs):
    q_nodes = np.asarray(inputs["q_nodes"], np.float32)
    k_edges = np.asarray(inputs["k_edges"], np.float32)
    v_edges = np.asarray(inputs["v_edges"], np.float32)
    Wq = np.asarray(inputs["Wq"], np.float32)
    bq = np.asarray(inputs["bq"], np.float32)
    Wk = np.asarray(inputs["Wk"], np.float32)
    Wv = np.asarray(inputs["Wv"], np.float32)
    bv = np.asarray(inputs["bv"], np.float32)
    Wo = np.asarray(inputs["Wo"], np.float32)
    bo = np.asarray(inputs["bo"], np.float32)
    dst = np.asarray(inputs["edge_index"])[0].astype(np.int64)

    plan = _make_plan(dst)

    eorder = np.argsort(dst, kind="stable")
    starts = np.zeros(N + 1, np.int64)
    np.cumsum(np.bincount(dst, minlength=N), out=starts[1:])
    edges_of = [eorder[starts[n]: starts[n + 1]] for n in range(N)]

    # M_h = B_h @ qT + c_h with B_h = WkS diag(Hm_h) Wq^T; P_h = Wv_h Wo_h
    WkS = Wk * SCALE
    Hm = (np.arange(DIM)[:, None] // DH == np.arange(HEADS)[None, :])
    BT4 = np.empty((DIM, HEADS * DIM), np.float32)
    c4 = np.empty((DIM, HEADS), np.float32)
    P4 = np.empty((DIM, HEADS * DIM), np.float32)
    for h in range(HEADS):
        m = Hm[:, h].astype(np.float32)
        BT4[:, h * DIM:(h + 1) * DIM] = (Wq * m[None, :]) @ WkS.T
        c4[:, h] = WkS @ (m * bq)
        hb = slice(h * DH, (h + 1) * DH)
        P4[:, h * DIM:(h + 1) * DIM] = Wv[:, hb] @ Wo[hb, :]

    consts = {
        "BT4": BT4.astype(np.float16),
        "c4": c4,
        "P4": P4,
        "ID": np.eye(DIM, dtype=np.float32),
        # sum(attn)==1 folds bv through Wo: out = agg@Wo + (bv@Wo + bo)
        "bo": (bv @ Wo + bo).reshape(DIM, 1).astype(np.float32),
    }
    return plan, dst, edges_of, consts, q_nodes, k_edges, v_edges, bo


def kernel(**inputs):
    from concourse.bass_utils import run_bass_kernel_spmd

    (plan, dst, edges_of, consts, q_nodes, k_edges, v_edges, bo) = _prepare(inputs)

    nc = _build_module(plan)

    in_maps = []
    slot_maps = []
    for c in range(NCORES):
        kvs, qT, qslot = _pack_core_inputs(plan, c, k_edges, v_edges,
                                           q_nodes, edges_of)
        m = {"kvs": kvs, "qT": qT}
        m.update(consts)
        in_maps.append(m)
        slot_maps.append(qslot)

    res = run_bass_kernel_spmd(nc, in_maps, core_ids=list(range(NCORES)))
    global LAST_RESULTS
    LAST_RESULTS = res

    out = np.zeros((N, DIM), np.float32)
    for c in range(NCORES):
        outT = res.results[c]["outT"]          # [DIM, nsp]
        qslot = slot_maps[c]
        valid = qslot >= 0
        out[qslot[valid]] = outT[:, : plan.nslot].T[valid]
    deg0 = plan.deg == 0
    if deg0.any():
        out[deg0] = bo
    return out
